# revision 13
# baseline (speedup 1.0000x reference)
"""NarrativeGraph GNN message-passing kernel for 8 Trainium2 NeuronCores.

Strategy (edge-sharded, per sharding hint):
  - E=50000 edges split 6250/core, padded to 6272 = 49 tiles of 128.
  - Per core: dma_gather src/dst node rows (1KB each) from HBM.
  - Relation-routed linear folded algebraically:
      sum_e edge_in[e] @ W[rel_e] = sum_r (S_r @ W_r),
      S_r = sum_{e: rel_e==r} concat(nodes[src_e], nodes[dst_e])
    computed as onehot(rel).T @ gathered_tile matmuls accumulating in PSUM.
  - 1KB AllReduce of the per-core partial graph state.
  - GRU (T=64 steps) + MLP replicated on every core, overlapping the gather.
"""

import sys

import numpy as np

_TRN = "/opt/trn_rl_repo"
if _TRN not in sys.path:
    sys.path.insert(0, _TRN)

H = 128
R = 8
N = 10000
E = 50000
B = 2
T = 64
NCORES = 8
EPC = E // NCORES          # 6250 edges per core
TILES = 49                 # ceil(6250/128)
EPAD = TILES * 128         # 6272
LN_EPS = 1e-5
GCHUNK = 7                 # gather chunk = 7 tiles = 896 edges
NCHUNKS = TILES // GCHUNK  # 7


def _wrap_idx(idx):
    """int16 wrapped layout for dma_gather: idx j at [j%16, j//16],
    replicated across the 8 gpsimd DSP stripes (partitions 16k..16k+15)."""
    w16 = idx.reshape(EPAD // 16, 16).T.astype(np.int16)
    return np.ascontiguousarray(np.tile(w16, (8, 1)))


def _build(nc, tc, flags):
    from concourse import bass
    import concourse.mybir as mybir

    f32 = mybir.dt.float32
    f32r = mybir.dt.float32r
    i16 = mybir.dt.int16
    Alu = mybir.AluOpType
    Act = mybir.ActivationFunctionType
    Axis = mybir.AxisListType

    # ---- DRAM I/O ----
    nodes_d = nc.dram_tensor("nodes_flat", [N, 2 * H], f32, kind="ExternalInput")
    sidx_d = nc.dram_tensor("sidx", [128, EPAD // 16], i16, kind="ExternalInput")
    didx_d = nc.dram_tensor("didx", [128, EPAD // 16], i16, kind="ExternalInput")
    relv_d = nc.dram_tensor("relv", [128, TILES], f32, kind="ExternalInput")
    relW_d = nc.dram_tensor("relW", [R, 2 * H, H], f32, kind="ExternalInput")
    tseq_d = nc.dram_tensor("tseq", [B * T, H], f32, kind="ExternalInput")
    wih_d = nc.dram_tensor("wih", [3 * H, H], f32, kind="ExternalInput")
    whh_d = nc.dram_tensor("whh", [3 * H, H], f32, kind="ExternalInput")
    w1_d = nc.dram_tensor("w1", [H, 2 * H], f32, kind="ExternalInput")
    w2_d = nc.dram_tensor("w2", [2 * H, H], f32, kind="ExternalInput")
    if flags["rel_b"]:
        relb_d = nc.dram_tensor("relb", [R, H], f32, kind="ExternalInput")
    if flags["bih"]:
        biht_d = nc.dram_tensor("biht", [H, 3], f32, kind="ExternalInput")
    if flags["bhh"]:
        bhht_d = nc.dram_tensor("bhht", [H, 3], f32, kind="ExternalInput")
    if flags["ln_g"]:
        lng_d = nc.dram_tensor("lng", [B, 2 * H], f32, kind="ExternalInput")
    if flags["ln_b"]:
        lnb_d = nc.dram_tensor("lnb", [B, 2 * H], f32, kind="ExternalInput")
    if flags["b1"]:
        b1_d = nc.dram_tensor("b1", [B, 2 * H], f32, kind="ExternalInput")
    if flags["b2"]:
        b2_d = nc.dram_tensor("b2", [B, H], f32, kind="ExternalInput")
    out_d = nc.dram_tensor("out", [B, H], f32, kind="ExternalOutput")

    from concourse.masks import make_identity
    from contextlib import ExitStack

    with ExitStack() as stk:
        pers = stk.enter_context(tc.tile_pool(name="pers", bufs=1))
        loop = stk.enter_context(tc.tile_pool(name="loop", bufs=3))
        pp = stk.enter_context(tc.tile_pool(name="pp", bufs=1, space="PSUM"))
        dram = stk.enter_context(tc.tile_pool(name="dram", bufs=1, space="DRAM"))

        # ---- persistent SBUF tiles ----
        ident = pers.tile([128, 128], f32, tag="ident")
        make_identity(nc, ident)

        gath = pers.tile([128, 2, TILES, 2 * H], f32r, tag="gath")
        oneh = pers.tile([128, TILES, R], f32, tag="oneh")
        oneh_r = pers.tile([128, TILES, R], f32r, tag="onehr")
        relv_sb = pers.tile([128, TILES], f32, tag="relv")
        iota_f = pers.tile([128, R], f32, tag="iota")
        sidx_sb = pers.tile([128, EPAD // 16], i16, tag="sidx")
        didx_sb = pers.tile([128, EPAD // 16], i16, tag="didx")
        relW_sb = pers.tile([128, R, 2, H], f32, tag="relW")
        ssb = pers.tile([R, 4 * H], f32, tag="ssb")
        st_sb = pers.tile([128, 2, B, R], f32, tag="st")
        xT = pers.tile([128, B, T], f32, tag="xT")
        wihT = pers.tile([128, 3, H], f32, tag="wihT")
        whhT = pers.tile([128, 3, H], f32, tag="whhT")
        gxT = pers.tile([128, 3, B, T], f32, tag="gxT")
        w1_sb = pers.tile([128, 2 * H], f32, tag="w1")
        w2_sb = pers.tile([128, 2, H], f32, tag="w2")
        gT_sb = pers.tile([128, B], f32, tag="gT")
        gfull = pers.tile([128, B], f32, tag="gfull")
        gsT = pers.tile([128, B], f32, tag="gsT")
        ones_sb = None
        if flags["rel_b"]:
            ones_sb = pers.tile([128, 1], f32, tag="ones")
            nc.vector.memset(ones_sb[:], 1.0)
            relb_sb = pers.tile([R, H], f32, tag="relb")
            nc.sync.dma_start(relb_sb[:], relb_d[:])
            cnt_sb = pers.tile([R, 1], f32, tag="cnt")

        # ---- input loads ----
        nc.sync.dma_start(sidx_sb[:], sidx_d[:])
        nc.sync.dma_start(didx_sb[:], didx_d[:])
        nc.sync.dma_start(relv_sb[:], relv_d[:])
        for r in range(R):
            for kc in range(2):
                nc.sync.dma_start(
                    relW_sb[:, r, kc, :], relW_d[r, kc * 128:(kc + 1) * 128, :]
                )
        nc.sync.dma_start(w1_sb[:], w1_d[:])
        for kc in range(2):
            nc.sync.dma_start(w2_sb[:, kc, :], w2_d[kc * 128:(kc + 1) * 128, :])

        # iota 0..R-1 along free dim (8 tiny memsets; avoids gpsimd iota lib)
        for r in range(R):
            nc.vector.memset(iota_f[:, r:r + 1], float(r))

        # onehot for all tiles in one DVE op: oneh[p,t,r] = (relv[p,t] == r)
        nc.vector.tensor_tensor(
            out=oneh[:],
            in0=relv_sb[:].unsqueeze(2).to_broadcast([128, TILES, R]),
            in1=iota_f[:].unsqueeze(1).to_broadcast([128, TILES, R]),
            op=Alu.is_equal,
        )
        nc.vector.tensor_copy(
            out=oneh_r[:].rearrange("p t r -> p (t r)"),
            in_=oneh[:].rearrange("p t r -> p (t r)"),
        )

        # ---- GRU setup: transposes + gx precompute ----
        xbt = pers.tile([128, H], f32, tag="xbt")
        nc.sync.dma_start(xbt[:], tseq_d[:])
        pt = pp.tile([128, 128], f32, tag="tp")
        nc.tensor.transpose(pt[:], xbt[:], ident[:])
        nc.vector.tensor_copy(out=xT[:].rearrange("p b t -> p (b t)"), in_=pt[:])

        wtmp = pers.tile([128, H], f32, tag="wtmp")
        for g in range(3):
            nc.sync.dma_start(wtmp[:], wih_d[g * 128:(g + 1) * 128, :])
            ptw = pp.tile([128, 128], f32, tag="tp")
            nc.tensor.transpose(ptw[:], wtmp[:], ident[:])
            nc.vector.tensor_copy(out=wihT[:, g, :], in_=ptw[:])
        wtmp2 = pers.tile([128, H], f32, tag="wtmp2")
        for g in range(3):
            nc.sync.dma_start(wtmp2[:], whh_d[g * 128:(g + 1) * 128, :])
            ptw = pp.tile([128, 128], f32, tag="tp")
            nc.tensor.transpose(ptw[:], wtmp2[:], ident[:])
            nc.vector.tensor_copy(out=whhT[:, g, :], in_=ptw[:])

        pgx = pp.tile([128, 3, B * T], f32, tag="gx")
        for g in range(3):
            nc.tensor.matmul(
                pgx[:, g, :], lhsT=wihT[:, g, :], rhs=xT[:].rearrange("p b t -> p (b t)"),
                start=True, stop=True,
            )
        nc.vector.tensor_copy(
            out=gxT[:].rearrange("p g b t -> p (g b t)"),
            in_=pgx[:].rearrange("p g n -> p (g n)"),
        )
        if flags["bih"]:
            biht_sb = pers.tile([128, 3], f32, tag="biht")
            nc.sync.dma_start(biht_sb[:], biht_d[:])
            nc.vector.tensor_tensor(
                out=gxT[:], in0=gxT[:],
                in1=biht_sb[:].unsqueeze(2).unsqueeze(3).to_broadcast([128, 3, B, T]),
                op=Alu.add,
            )
        if flags["bhh"]:
            bhht_sb = pers.tile([128, 3], f32, tag="bhht")
            nc.sync.dma_start(bhht_sb[:], bhht_d[:])
            nc.vector.tensor_tensor(
                out=gxT[:], in0=gxT[:],
                in1=bhht_sb[:].unsqueeze(2).unsqueeze(3).to_broadcast([128, 3, B, T]),
                op=Alu.add,
            )

        # ---- issue all gathers (SWDGE, overlapped with GRU) ----
        ecols = GCHUNK * 128 // 16  # idx cols per chunk
        for c in range(NCHUNKS):
            nc.gpsimd.dma_gather(
                gath[:, 0, c * GCHUNK:(c + 1) * GCHUNK, :],
                nodes_d[:].bitcast(f32r),
                sidx_sb[:, c * ecols:(c + 1) * ecols],
                GCHUNK * 128, GCHUNK * 128, 2 * H,
                queue_num=0,
            )
            nc.gpsimd.dma_gather(
                gath[:, 1, c * GCHUNK:(c + 1) * GCHUNK, :],
                nodes_d[:].bitcast(f32r),
                didx_sb[:, c * ecols:(c + 1) * ecols],
                GCHUNK * 128, GCHUNK * 128, 2 * H,
                queue_num=0,
            )

        # ---- S accumulation psum (lives across GRU loop) ----
        psum_s = pp.tile([R, 4 * H], f32, tag="S")

        def s_matmuls(tlo, thi):
            for t in range(tlo, thi):
                # rhs free order (b, s, col): matches W row layout [src|dst] per b
                rhs = gath[:, :, t, :].rearrange("p s (b c) -> p b s c", b=B)
                nc.tensor.matmul(
                    psum_s[:],
                    lhsT=oneh_r[:, t, :],
                    rhs=rhs,
                    start=(t == 0), stop=(t == TILES - 1),
                )
            if flags["rel_b"] and thi == TILES:
                pcnt = pp.tile([R, 1], f32, tag="cntp")
                for t in range(TILES):
                    nc.tensor.matmul(
                        pcnt[:], lhsT=oneh[:, t, :], rhs=ones_sb[:],
                        start=(t == 0), stop=(t == TILES - 1),
                    )
                nc.vector.tensor_copy(out=cnt_sb[:], in_=pcnt[:])

        # ---- GRU loop with S-matmul chunks interleaved ----
        h_prev = loop.tile([128, B], f32, tag="h", bufs=2)
        nc.vector.memset(h_prev[:], 0.0)
        # schedule S chunk c after GRU step 9*(c+1)
        s_after = {9 * (c + 1): c for c in range(NCHUNKS)}
        done_chunks = 0
        for t in range(T):
            pgh = pp.tile([128, 3, B], f32, tag="gh", bufs=2)
            for g in range(3):
                nc.tensor.matmul(
                    pgh[:, g, :], lhsT=whhT[:, g, :], rhs=h_prev[:],
                    start=True, stop=True,
                )
            # r/z gates: sigma(gx + gh); n gate: tanh(gx_n + r*gh_n)
            # (r multiplies only the hidden term, per torch GRU)
            gs = loop.tile([128, 2, B], f32, tag="gs", bufs=3)
            nc.vector.tensor_tensor(
                out=gs[:], in0=pgh[:, 0:2, :], in1=gxT[:, 0:2, :, t], op=Alu.add
            )
            rz = loop.tile([128, 2, B], f32, tag="rz", bufs=3)
            nc.scalar.activation(rz[:], gs[:], Act.Sigmoid)
            hn = pgh[:, 2, :]
            if flags["bhh"]:
                hnb = loop.tile([128, B], f32, tag="hnb", bufs=3)
                nc.vector.tensor_tensor(
                    out=hnb[:], in0=pgh[:, 2, :],
                    in1=bhht_sb[:, 2:3].to_broadcast([128, B]),
                    op=Alu.add,
                )
                hn = hnb[:]
            rn = loop.tile([128, B], f32, tag="rn", bufs=3)
            nc.vector.tensor_tensor(out=rn[:], in0=rz[:, 0, :], in1=hn, op=Alu.mult)
            nc.vector.tensor_tensor(
                out=rn[:], in0=rn[:], in1=gxT[:, 2, :, t], op=Alu.add
            )
            z_t = rz[:, 1, :]
            s_t = loop.tile([128, B], f32, tag="s", bufs=3)
            nc.scalar.activation(s_t[:], rn[:], Act.Sigmoid)
            n_t = loop.tile([128, B], f32, tag="n", bufs=3)
            nc.vector.tensor_scalar(n_t[:], s_t[:], 2.0, -1.0, Alu.mult, Alu.add)
            d_t = loop.tile([128, B], f32, tag="d", bufs=3)
            nc.vector.tensor_tensor(out=d_t[:], in0=h_prev[:], in1=n_t[:], op=Alu.subtract)
            nc.vector.tensor_tensor(out=d_t[:], in0=z_t, in1=d_t[:], op=Alu.mult)
            h_new = loop.tile([128, B], f32, tag="h", bufs=2)
            nc.vector.tensor_tensor(out=h_new[:], in0=n_t[:], in1=d_t[:], op=Alu.add)
            h_prev = h_new
            if t in s_after:
                c = s_after[t]
                s_matmuls(c * GCHUNK, (c + 1) * GCHUNK)
                done_chunks = c + 1
        if done_chunks < NCHUNKS:
            s_matmuls(done_chunks * GCHUNK, TILES)

        # ---- S -> G partial: transpose S then contract with rel_W ----
        nc.vector.tensor_copy(out=ssb[:], in_=psum_s[:])
        ptS = pp.tile([128, 2 * B * R], f32, tag="tp")
        for b in range(B):
            for kc in range(2):
                nc.tensor.transpose(
                    ptS[:, (kc * B + b) * R:(kc * B + b + 1) * R],
                    ssb[:, b * 2 * H + kc * H: b * 2 * H + (kc + 1) * H],
                    ident[:8, :8],
                )
        nc.vector.tensor_copy(
            out=st_sb[:].rearrange("p k b r -> p (k b r)"), in_=ptS[:]
        )

        pG = pp.tile([128, B], f32, tag="G")
        nmm = 2 * R + (1 if flags["rel_b"] else 0)
        i = 0
        for kc in range(2):
            for r in range(R):
                nc.tensor.matmul(
                    pG[:], lhsT=relW_sb[:, r, kc, :], rhs=st_sb[:, kc, :, r],
                    start=(i == 0), stop=(i == nmm - 1),
                )
                i += 1
        if flags["rel_b"]:
            nc.tensor.matmul(
                pG[:], lhsT=relb_sb[:], rhs=cnt_sb[:].to_broadcast([R, B]),
                start=False, stop=True,
            )
        nc.vector.tensor_copy(out=gT_sb[:], in_=pG[:])

        # ---- AllReduce the 1KB partial over the 8 cores ----
        cc_in = dram.tile([128, B], f32, tag="ccin")
        cc_out = dram.tile([128, B], f32, tag="ccout")
        nc.gpsimd.dma_start(cc_in[:], gT_sb[:])
        nc.gpsimd.collective_compute(
            "AllReduce",
            Alu.add,
            replica_groups=[list(range(NCORES))],
            ins=[cc_in[:]],
            outs=[cc_out[:]],
        )
        nc.gpsimd.dma_start(gfull[:], cc_out[:])

        # graph_state^T = G/E + h_last
        nc.vector.tensor_scalar(gsT[:], gfull[:], 1.0 / E, None, Alu.mult)
        nc.vector.tensor_tensor(out=gsT[:], in0=gsT[:], in1=h_prev[:], op=Alu.add)

        # ---- MLP + LayerNorm (row layout [B, .]) ----
        ph1 = pp.tile([B, 2 * H], f32, tag="mlp")
        nc.tensor.matmul(
            ph1[:], lhsT=gsT[:], rhs=w1_sb[:], start=True, stop=True,
        )
        x_sb = pers.tile([B, 2 * H], f32, tag="xsb")
        if flags["b1"]:
            b1_sb = pers.tile([B, 2 * H], f32, tag="b1")
            nc.sync.dma_start(b1_sb[:], b1_d[:])
            nc.vector.tensor_tensor(out=x_sb[:], in0=ph1[:], in1=b1_sb[:], op=Alu.add)
        else:
            nc.vector.tensor_copy(out=x_sb[:], in_=ph1[:])
        mu = pers.tile([B, 1], f32, tag="mu")
        nc.vector.tensor_reduce(out=mu[:], in_=x_sb[:], axis=Axis.X, op=Alu.add)
        nc.vector.tensor_scalar(mu[:], mu[:], 1.0 / (2 * H), None, Alu.mult)
        xc = pers.tile([B, 2 * H], f32, tag="xc")
        nc.vector.tensor_scalar(xc[:], x_sb[:], mu[:], None, Alu.subtract)
        sq = pers.tile([B, 2 * H], f32, tag="sq")
        var = pers.tile([B, 1], f32, tag="var")
        nc.scalar.activation(sq[:], xc[:], Act.Square, accum_out=var[:])
        sd = pers.tile([B, 1], f32, tag="sd")
        eps_t = pers.tile([B, 1], f32, tag="eps")
        nc.vector.memset(eps_t[:], LN_EPS)
        nc.scalar.activation(sd[:], var[:], Act.Sqrt, bias=eps_t[:], scale=1.0 / (2 * H))
        rs = pers.tile([B, 1], f32, tag="rs")
        nc.vector.reciprocal(out=rs[:], in_=sd[:])
        y = pers.tile([B, 2 * H], f32, tag="y")
        nc.vector.tensor_scalar(y[:], xc[:], rs[:], None, Alu.mult)
        if flags["ln_g"]:
            lng_sb = pers.tile([B, 2 * H], f32, tag="lng")
            nc.sync.dma_start(lng_sb[:], lng_d[:])
            nc.vector.tensor_tensor(out=y[:], in0=y[:], in1=lng_sb[:], op=Alu.mult)
        if flags["ln_b"]:
            lnb_sb = pers.tile([B, 2 * H], f32, tag="lnb")
            nc.sync.dma_start(lnb_sb[:], lnb_d[:])
            nc.vector.tensor_tensor(out=y[:], in0=y[:], in1=lnb_sb[:], op=Alu.add)
        h1_sb = pers.tile([B, 2 * H], f32, tag="h1")
        nc.scalar.activation(h1_sb[:], y[:], Act.Relu)

        # h1^T via PE transpose, then final matmul
        h1T = pers.tile([128, 2, B], f32, tag="h1T")
        pth = pp.tile([128, 2 * B], f32, tag="tp")
        for kc in range(2):
            nc.tensor.transpose(
                pth[:, kc * B:(kc + 1) * B],
                h1_sb[:, kc * 128:(kc + 1) * 128],
                ident[:B, :B],
            )
        nc.vector.tensor_copy(out=h1T[:].rearrange("p k b -> p (k b)"), in_=pth[:])

        po = pp.tile([B, H], f32, tag="gx")
        for kc in range(2):
            nc.tensor.matmul(
                po[:], lhsT=h1T[:, kc, :], rhs=w2_sb[:, kc, :],
                start=(kc == 0), stop=(kc == 1),
            )
        o_sb = pers.tile([B, H], f32, tag="osb")
        if flags["b2"]:
            b2_sb = pers.tile([B, H], f32, tag="b2")
            nc.sync.dma_start(b2_sb[:], b2_d[:])
            nc.vector.tensor_tensor(out=o_sb[:], in0=po[:], in1=b2_sb[:], op=Alu.add)
        else:
            nc.vector.tensor_copy(out=o_sb[:], in_=po[:])
        nc.sync.dma_start(out_d[:], o_sb[:])


def build_kernel(inputs):
    """Trace + compile; returns (nc, in_maps)."""
    from concourse import bacc, tile

    nodes = np.ascontiguousarray(
        np.asarray(inputs["nodes"], dtype=np.float32).reshape(N, 2 * H)
    )
    src = np.asarray(inputs["src"]).astype(np.int64)
    rel = np.asarray(inputs["rel"]).astype(np.int64)
    dst = np.asarray(inputs["dst"]).astype(np.int64)
    rel_W = np.ascontiguousarray(np.asarray(inputs["rel_W"], dtype=np.float32))
    rel_b = np.asarray(inputs["rel_b"], dtype=np.float32)
    gru_Wih = np.asarray(inputs["gru_Wih"], dtype=np.float32)
    gru_Whh = np.asarray(inputs["gru_Whh"], dtype=np.float32)
    gru_bih = np.asarray(inputs["gru_bih"], dtype=np.float32)
    gru_bhh = np.asarray(inputs["gru_bhh"], dtype=np.float32)
    mlp_W1 = np.ascontiguousarray(np.asarray(inputs["mlp_W1"], dtype=np.float32))
    mlp_b1 = np.asarray(inputs["mlp_b1"], dtype=np.float32)
    ln_g = np.asarray(inputs["ln_g"], dtype=np.float32)
    ln_b = np.asarray(inputs["ln_b"], dtype=np.float32)
    mlp_W2 = np.ascontiguousarray(np.asarray(inputs["mlp_W2"], dtype=np.float32))
    mlp_b2 = np.asarray(inputs["mlp_b2"], dtype=np.float32)
    tseq = np.ascontiguousarray(
        np.asarray(inputs["temporal_sequence"], dtype=np.float32).reshape(B * T, H)
    )

    # sigmoid-only GRU: tanh(x) = 2*sigmoid(2x) - 1, fold the 2x into n-gate
    # weights/biases (rows 2H:3H)
    wih2 = gru_Wih.copy()
    wih2[2 * H:] *= 2.0
    whh2 = gru_Whh.copy()
    whh2[2 * H:] *= 2.0
    bih2 = gru_bih.copy()
    bih2[2 * H:] *= 2.0
    bhh2 = gru_bhh.copy()
    bhh2[2 * H:] *= 2.0

    flags = {
        "rel_b": bool(np.any(rel_b != 0)),
        "bih": bool(np.any(bih2 != 0)),
        "bhh": bool(np.any(bhh2 != 0)),
        "ln_g": bool(np.any(ln_g != 1)),
        "ln_b": bool(np.any(ln_b != 0)),
        "b1": bool(np.any(mlp_b1 != 0)),
        "b2": bool(np.any(mlp_b2 != 0)),
    }

    nc = bacc.Bacc(
        "TRN2", target_bir_lowering=False, debug=False, num_devices=NCORES
    )
    with tile.TileContext(nc) as tc:
        _build(nc, tc, flags)
    nc.compile()

    in_maps = []
    for c in range(NCORES):
        lo, hi = c * EPC, (c + 1) * EPC
        s_pad = np.zeros(EPAD, np.int64)
        d_pad = np.zeros(EPAD, np.int64)
        r_pad = np.full(EPAD, R, np.int64)  # rel=R -> zero onehot row
        s_pad[:EPC] = src[lo:hi]
        d_pad[:EPC] = dst[lo:hi]
        r_pad[:EPC] = rel[lo:hi]
        m = {
            "nodes_flat": nodes,
            "sidx": _wrap_idx(s_pad),
            "didx": _wrap_idx(d_pad),
            "relv": np.ascontiguousarray(
                r_pad.reshape(TILES, 128).T.astype(np.float32)
            ),
            "relW": rel_W,
            "tseq": tseq,
            "wih": wih2,
            "whh": whh2,
            "w1": mlp_W1,
            "w2": mlp_W2,
        }
        if flags["rel_b"]:
            m["relb"] = rel_b
        if flags["bih"]:
            m["biht"] = np.ascontiguousarray(bih2.reshape(3, H).T)
        if flags["bhh"]:
            m["bhht"] = np.ascontiguousarray(bhh2.reshape(3, H).T)
        if flags["ln_g"]:
            m["lng"] = np.ascontiguousarray(np.broadcast_to(ln_g, (B, 2 * H)))
        if flags["ln_b"]:
            m["lnb"] = np.ascontiguousarray(np.broadcast_to(ln_b, (B, 2 * H)))
        if flags["b1"]:
            m["b1"] = np.ascontiguousarray(np.broadcast_to(mlp_b1, (B, 2 * H)))
        if flags["b2"]:
            m["b2"] = np.ascontiguousarray(np.broadcast_to(mlp_b2, (B, H)))
        in_maps.append(m)
    return nc, in_maps


def run(inputs, trace=False):
    from concourse import bass_utils

    nc, in_maps = build_kernel(inputs)
    res = bass_utils.run_bass_kernel_spmd(
        nc, in_maps, core_ids=list(range(NCORES)), trace=trace
    )
    return res


def kernel(**inputs):
    res = run(inputs, trace=False)
    return np.asarray(res.results[0]["out"], dtype=np.float32)


# revision 16
# speedup vs baseline: 1.4799x; 1.4799x over previous
"""NarrativeGraph GNN message-passing kernel for 8 Trainium2 NeuronCores.

Strategy (edge-sharded, per sharding hint):
  - E=50000 edges split 6250/core, padded to 6272 = 49 tiles of 128.
  - Per core: dma_gather src/dst node rows (1KB each) from HBM.
  - Relation-routed linear folded algebraically:
      sum_e edge_in[e] @ W[rel_e] = sum_r (S_r @ W_r),
      S_r = sum_{e: rel_e==r} concat(nodes[src_e], nodes[dst_e])
    computed as onehot(rel).T @ gathered_tile matmuls accumulating in PSUM.
  - 1KB AllReduce of the per-core partial graph state.
  - GRU (T=64 steps) + MLP replicated on every core, overlapping the gather.
"""

import sys

import numpy as np

_TRN = "/opt/trn_rl_repo"
if _TRN not in sys.path:
    sys.path.insert(0, _TRN)

H = 128
R = 8
N = 10000
E = 50000
B = 2
T = 64
NCORES = 8
EPC = E // NCORES          # 6250 edges per core
TILES = 49                 # ceil(6250/128)
EPAD = TILES * 128         # 6272
LN_EPS = 1e-5
GCHUNK = 7                 # gather chunk = 7 tiles = 896 edges
NCHUNKS = TILES // GCHUNK  # 7


def _wrap_idx(idx):
    """int16 wrapped layout for dma_gather: idx j at [j%16, j//16],
    replicated across the 8 gpsimd DSP stripes (partitions 16k..16k+15)."""
    w16 = idx.reshape(EPAD // 16, 16).T.astype(np.int16)
    return np.ascontiguousarray(np.tile(w16, (8, 1)))


def _build(nc, tc, flags):
    from concourse import bass
    import concourse.mybir as mybir

    f32 = mybir.dt.float32
    f32r = mybir.dt.float32r
    i16 = mybir.dt.int16
    Alu = mybir.AluOpType
    Act = mybir.ActivationFunctionType
    Axis = mybir.AxisListType

    # ---- DRAM I/O ----
    nodes_d = nc.dram_tensor("nodes_flat", [N, 2 * H], f32, kind="ExternalInput")
    sidx_d = nc.dram_tensor("sidx", [128, EPAD // 16], i16, kind="ExternalInput")
    didx_d = nc.dram_tensor("didx", [128, EPAD // 16], i16, kind="ExternalInput")
    relv_d = nc.dram_tensor("relv", [128, TILES], f32, kind="ExternalInput")
    relW_d = nc.dram_tensor("relW", [R, 2 * H, H], f32, kind="ExternalInput")
    tseq_d = nc.dram_tensor("tseq", [B * T, H], f32, kind="ExternalInput")
    wih_d = nc.dram_tensor("wih", [3 * H, H], f32, kind="ExternalInput")
    whh_d = nc.dram_tensor("whh", [3 * H, H], f32, kind="ExternalInput")
    w1_d = nc.dram_tensor("w1", [H, 2 * H], f32, kind="ExternalInput")
    w2_d = nc.dram_tensor("w2", [2 * H, H], f32, kind="ExternalInput")
    if flags["rel_b"]:
        relb_d = nc.dram_tensor("relb", [R, H], f32, kind="ExternalInput")
    if flags["bih"]:
        biht_d = nc.dram_tensor("biht", [H, 3], f32, kind="ExternalInput")
    if flags["bhh"]:
        bhht_d = nc.dram_tensor("bhht", [H, 3], f32, kind="ExternalInput")
    if flags["ln_g"]:
        lng_d = nc.dram_tensor("lng", [B, 2 * H], f32, kind="ExternalInput")
    if flags["ln_b"]:
        lnb_d = nc.dram_tensor("lnb", [B, 2 * H], f32, kind="ExternalInput")
    if flags["b1"]:
        b1_d = nc.dram_tensor("b1", [B, 2 * H], f32, kind="ExternalInput")
    if flags["b2"]:
        b2_d = nc.dram_tensor("b2", [B, H], f32, kind="ExternalInput")
    out_d = nc.dram_tensor("out", [B, H], f32, kind="ExternalOutput")

    from concourse.masks import make_identity
    from contextlib import ExitStack

    with ExitStack() as stk:
        pers = stk.enter_context(tc.tile_pool(name="pers", bufs=1))
        loop = stk.enter_context(tc.tile_pool(name="loop", bufs=3))
        pp = stk.enter_context(tc.tile_pool(name="pp", bufs=1, space="PSUM"))
        dram = stk.enter_context(tc.tile_pool(name="dram", bufs=1, space="DRAM"))

        # ---- persistent SBUF tiles ----
        ident = pers.tile([128, 128], f32, tag="ident")
        make_identity(nc, ident)

        gath = pers.tile([128, 2, TILES, 2 * H], f32r, tag="gath")
        oneh = pers.tile([128, TILES, R], f32, tag="oneh")
        oneh_r = pers.tile([128, TILES, R], f32r, tag="onehr")
        relv_sb = pers.tile([128, TILES], f32, tag="relv")
        iota_f = pers.tile([128, R], f32, tag="iota")
        sidx_sb = pers.tile([128, EPAD // 16], i16, tag="sidx")
        didx_sb = pers.tile([128, EPAD // 16], i16, tag="didx")
        relW_sb = pers.tile([128, R, 2, H], f32, tag="relW")
        ssb = pers.tile([R, 4 * H], f32, tag="ssb")
        st_sb = pers.tile([128, 2, B, R], f32, tag="st")
        xT = pers.tile([128, B, T], f32, tag="xT")
        wihT = pers.tile([128, 3, H], f32, tag="wihT")
        whhT = pers.tile([128, 3, H], f32, tag="whhT")
        gxT = pers.tile([128, 3, B, T], f32, tag="gxT")
        w1_sb = pers.tile([128, 2 * H], f32, tag="w1")
        w2_sb = pers.tile([128, 2, H], f32, tag="w2")
        gT_sb = pers.tile([128, B], f32, tag="gT")
        gfull = pers.tile([128, B], f32, tag="gfull")
        gsT = pers.tile([128, B], f32, tag="gsT")
        ones_sb = None
        if flags["rel_b"]:
            ones_sb = pers.tile([128, 1], f32, tag="ones")
            nc.vector.memset(ones_sb[:], 1.0)
            relb_sb = pers.tile([R, H], f32, tag="relb")
            nc.sync.dma_start(relb_sb[:], relb_d[:])
            cnt_sb = pers.tile([R, 1], f32, tag="cnt")

        # ---- input loads ----
        nc.sync.dma_start(sidx_sb[:], sidx_d[:])
        nc.sync.dma_start(didx_sb[:], didx_d[:])
        nc.sync.dma_start(relv_sb[:], relv_d[:])
        for r in range(R):
            for kc in range(2):
                nc.sync.dma_start(
                    relW_sb[:, r, kc, :], relW_d[r, kc * 128:(kc + 1) * 128, :]
                )
        nc.sync.dma_start(w1_sb[:], w1_d[:])
        for kc in range(2):
            nc.sync.dma_start(w2_sb[:, kc, :], w2_d[kc * 128:(kc + 1) * 128, :])

        # iota 0..R-1 along free dim (8 tiny memsets; avoids gpsimd iota lib)
        for r in range(R):
            nc.vector.memset(iota_f[:, r:r + 1], float(r))

        # onehot for all tiles in one DVE op: oneh[p,t,r] = (relv[p,t] == r)
        nc.vector.tensor_tensor(
            out=oneh[:],
            in0=relv_sb[:].unsqueeze(2).to_broadcast([128, TILES, R]),
            in1=iota_f[:].unsqueeze(1).to_broadcast([128, TILES, R]),
            op=Alu.is_equal,
        )
        nc.vector.tensor_copy(
            out=oneh_r[:].rearrange("p t r -> p (t r)"),
            in_=oneh[:].rearrange("p t r -> p (t r)"),
        )

        # ---- GRU setup: transposes + gx precompute ----
        xbt = pers.tile([128, H], f32, tag="xbt")
        nc.sync.dma_start(xbt[:], tseq_d[:])
        pt = pp.tile([128, 128], f32, tag="tp")
        nc.tensor.transpose(pt[:], xbt[:], ident[:])
        nc.vector.tensor_copy(out=xT[:].rearrange("p b t -> p (b t)"), in_=pt[:])

        wtmp = pers.tile([128, H], f32, tag="wtmp")
        for g in range(3):
            nc.sync.dma_start(wtmp[:], wih_d[g * 128:(g + 1) * 128, :])
            ptw = pp.tile([128, 128], f32, tag="tp")
            nc.tensor.transpose(ptw[:], wtmp[:], ident[:])
            nc.vector.tensor_copy(out=wihT[:, g, :], in_=ptw[:])
        wtmp2 = pers.tile([128, H], f32, tag="wtmp2")
        for g in range(3):
            nc.sync.dma_start(wtmp2[:], whh_d[g * 128:(g + 1) * 128, :])
            ptw = pp.tile([128, 128], f32, tag="tp")
            nc.tensor.transpose(ptw[:], wtmp2[:], ident[:])
            nc.vector.tensor_copy(out=whhT[:, g, :], in_=ptw[:])

        pgx = pp.tile([128, 3, B * T], f32, tag="gx")
        for g in range(3):
            nc.tensor.matmul(
                pgx[:, g, :], lhsT=wihT[:, g, :], rhs=xT[:].rearrange("p b t -> p (b t)"),
                start=True, stop=True,
            )
        nc.vector.tensor_copy(
            out=gxT[:].rearrange("p g b t -> p (g b t)"),
            in_=pgx[:].rearrange("p g n -> p (g n)"),
        )
        # gxz = gxT with the n-gate zeroed: per-step PSUM inject for r/z
        # (gx_n enters via the ACT bias instead, since pgh[2] must be hn only)
        gxz = pers.tile([128, 3, B, T], f32, tag="gxz")
        nc.vector.tensor_copy(
            out=gxz[:, 0:2].rearrange("p g b t -> p (g b t)"),
            in_=gxT[:, 0:2].rearrange("p g b t -> p (g b t)"),
        )
        nc.vector.memset(gxz[:, 2].rearrange("p b t -> p (b t)"), 0.0)
        zstop = pers.tile([128, 3 * B], f32, tag="zstop")
        nc.vector.memset(zstop[:], 0.0)
        if flags["bih"]:
            biht_sb = pers.tile([128, 3], f32, tag="biht")
            nc.sync.dma_start(biht_sb[:], biht_d[:])
            nc.vector.tensor_tensor(
                out=gxT[:], in0=gxT[:],
                in1=biht_sb[:].unsqueeze(2).unsqueeze(3).to_broadcast([128, 3, B, T]),
                op=Alu.add,
            )
        if flags["bhh"]:
            bhht_sb = pers.tile([128, 3], f32, tag="bhht")
            nc.sync.dma_start(bhht_sb[:], bhht_d[:])
            nc.vector.tensor_tensor(
                out=gxT[:], in0=gxT[:],
                in1=bhht_sb[:].unsqueeze(2).unsqueeze(3).to_broadcast([128, 3, B, T]),
                op=Alu.add,
            )

        # ---- issue all gathers (SWDGE, overlapped with GRU) ----
        ecols = GCHUNK * 128 // 16  # idx cols per chunk
        for c in range(NCHUNKS):
            nc.gpsimd.dma_gather(
                gath[:, 0, c * GCHUNK:(c + 1) * GCHUNK, :],
                nodes_d[:].bitcast(f32r),
                sidx_sb[:, c * ecols:(c + 1) * ecols],
                GCHUNK * 128, GCHUNK * 128, 2 * H,
                queue_num=0,
            )
            nc.gpsimd.dma_gather(
                gath[:, 1, c * GCHUNK:(c + 1) * GCHUNK, :],
                nodes_d[:].bitcast(f32r),
                didx_sb[:, c * ecols:(c + 1) * ecols],
                GCHUNK * 128, GCHUNK * 128, 2 * H,
                queue_num=0,
            )

        # ---- S accumulation psum (lives across GRU loop) ----
        psum_s = pp.tile([R, 4 * H], f32, tag="S")

        def s_matmuls(tlo, thi):
            for t in range(tlo, thi):
                # rhs free order (b, s, col): matches W row layout [src|dst] per b
                rhs = gath[:, :, t, :].rearrange("p s (b c) -> p b s c", b=B)
                nc.tensor.matmul(
                    psum_s[:],
                    lhsT=oneh_r[:, t, :],
                    rhs=rhs,
                    start=(t == 0), stop=(t == TILES - 1),
                )
            if flags["rel_b"] and thi == TILES:
                pcnt = pp.tile([R, 1], f32, tag="cntp")
                for t in range(TILES):
                    nc.tensor.matmul(
                        pcnt[:], lhsT=oneh[:, t, :], rhs=ones_sb[:],
                        start=(t == 0), stop=(t == TILES - 1),
                    )
                nc.vector.tensor_copy(out=cnt_sb[:], in_=pcnt[:])

        # ---- GRU loop (critical chain: PE -> ACT -> DVE -> PE) ----
        # pgh[0:2] accumulates gx_{r,z} (identity inject) + Whh_{r,z} h in
        # PSUM so sigma reads it directly; n gate fused on ACT as
        # sigmoid(r*hn + gx_n) via scale/bias APs; blend uses
        # h_new = q*w2 + c with w2 = 2(1-z), c = z*(h_prev+1) - 1 where
        # w2/c are computed off the critical chain.
        h_prev = loop.tile([128, B], f32, tag="h", bufs=2)
        nc.vector.memset(h_prev[:], 0.0)
        hp1 = loop.tile([128, B], f32, tag="hp1", bufs=2)
        nc.vector.memset(hp1[:], 1.0)
        for t in range(T):
            pgh = pp.tile([128, 3, B], f32, tag="gh", bufs=2)
            # one zero region per PSUM bank: single start (inject, zeroes all
            # of pgh), unflagged accumulates, single stop (zero matmul)
            nc.tensor.matmul(
                pgh[:], lhsT=ident[:], rhs=gxz[:, :, :, t],
                start=True, stop=False,
            )
            for g in range(3):
                nc.tensor.matmul(
                    pgh[:, g, :], lhsT=whhT[:, g, :], rhs=h_prev[:],
                    start=False, stop=False,
                )
            nc.tensor.matmul(
                pgh[:].rearrange("p g b -> p (g b)"), lhsT=ident[:], rhs=zstop[:],
                start=False, stop=True,
            )
            rz = loop.tile([128, 2, B], f32, tag="rz", bufs=3)
            nc.scalar.activation(rz[:], pgh[:, 0:2, :], Act.Sigmoid)
            hnb = None
            if flags["bhh"]:
                hnb = loop.tile([128, B], f32, tag="hnb", bufs=3)
                nc.vector.tensor_tensor(
                    out=hnb[:], in0=pgh[:, 2, :],
                    in1=bhht_sb[:, 2:3].to_broadcast([128, B]),
                    op=Alu.add,
                )
            q_t = loop.tile([128, B], f32, tag="q", bufs=3)
            for b in range(B):
                src = hnb[:, b:b + 1] if hnb is not None else pgh[:, 2, b:b + 1]
                nc.scalar.activation(
                    q_t[:, b:b + 1], src, Act.Sigmoid,
                    bias=gxT[:, 2, b:b + 1, t], scale=rz[:, 0, b:b + 1],
                )
            w2t = loop.tile([128, B], f32, tag="w2t", bufs=3)
            nc.vector.tensor_scalar(w2t[:], rz[:, 1, :], -2.0, 2.0, Alu.mult, Alu.add)
            zh1 = loop.tile([128, B], f32, tag="zh1", bufs=3)
            nc.vector.tensor_tensor(out=zh1[:], in0=rz[:, 1, :], in1=hp1[:], op=Alu.mult)
            cbl = loop.tile([128, B], f32, tag="cbl", bufs=3)
            nc.vector.tensor_scalar(cbl[:], zh1[:], -1.0, None, Alu.add)
            t1 = loop.tile([128, B], f32, tag="t1", bufs=3)
            nc.vector.tensor_tensor(out=t1[:], in0=q_t[:], in1=w2t[:], op=Alu.mult)
            h_new = loop.tile([128, B], f32, tag="h", bufs=2)
            nc.vector.tensor_tensor(out=h_new[:], in0=t1[:], in1=cbl[:], op=Alu.add)
            hp1 = loop.tile([128, B], f32, tag="hp1", bufs=2)
            nc.vector.tensor_scalar(hp1[:], h_new[:], 1.0, None, Alu.add)
            h_prev = h_new

        # S-matmul chunks: wait_until stamps keep the scheduler from placing
        # them before their gathers land (it models gathers as ~instant),
        # so the GRU chain claims the PE from the start.
        for c in range(NCHUNKS):
            with tc.tile_wait_until(0.0065 + 0.0032 * c):
                s_matmuls(c * GCHUNK, (c + 1) * GCHUNK)

        # ---- S -> G partial: transpose S then contract with rel_W ----
        nc.vector.tensor_copy(out=ssb[:], in_=psum_s[:])
        ptS = pp.tile([128, 2 * B * R], f32, tag="tp")
        for b in range(B):
            for kc in range(2):
                nc.tensor.transpose(
                    ptS[:, (kc * B + b) * R:(kc * B + b + 1) * R],
                    ssb[:, b * 2 * H + kc * H: b * 2 * H + (kc + 1) * H],
                    ident[:8, :8],
                )
        nc.vector.tensor_copy(
            out=st_sb[:].rearrange("p k b r -> p (k b r)"), in_=ptS[:]
        )

        pG = pp.tile([128, B], f32, tag="G")
        nmm = 2 * R + (1 if flags["rel_b"] else 0)
        i = 0
        for kc in range(2):
            for r in range(R):
                nc.tensor.matmul(
                    pG[:], lhsT=relW_sb[:, r, kc, :], rhs=st_sb[:, kc, :, r],
                    start=(i == 0), stop=(i == nmm - 1),
                )
                i += 1
        if flags["rel_b"]:
            nc.tensor.matmul(
                pG[:], lhsT=relb_sb[:], rhs=cnt_sb[:].to_broadcast([R, B]),
                start=False, stop=True,
            )
        nc.vector.tensor_copy(out=gT_sb[:], in_=pG[:])

        # ---- AllReduce the 1KB partial over the 8 cores ----
        cc_in = dram.tile([128, B], f32, tag="ccin")
        cc_out = dram.tile([128, B], f32, tag="ccout")
        nc.gpsimd.dma_start(cc_in[:], gT_sb[:])
        nc.gpsimd.collective_compute(
            "AllReduce",
            Alu.add,
            replica_groups=[list(range(NCORES))],
            ins=[cc_in[:]],
            outs=[cc_out[:]],
        )
        nc.gpsimd.dma_start(gfull[:], cc_out[:])

        # graph_state^T = G/E + h_last
        nc.vector.tensor_scalar(gsT[:], gfull[:], 1.0 / E, None, Alu.mult)
        nc.vector.tensor_tensor(out=gsT[:], in0=gsT[:], in1=h_prev[:], op=Alu.add)

        # ---- MLP + LayerNorm (row layout [B, .]) ----
        ph1 = pp.tile([B, 2 * H], f32, tag="mlp")
        nc.tensor.matmul(
            ph1[:], lhsT=gsT[:], rhs=w1_sb[:], start=True, stop=True,
        )
        x_sb = pers.tile([B, 2 * H], f32, tag="xsb")
        if flags["b1"]:
            b1_sb = pers.tile([B, 2 * H], f32, tag="b1")
            nc.sync.dma_start(b1_sb[:], b1_d[:])
            nc.vector.tensor_tensor(out=x_sb[:], in0=ph1[:], in1=b1_sb[:], op=Alu.add)
        else:
            nc.vector.tensor_copy(out=x_sb[:], in_=ph1[:])
        mu = pers.tile([B, 1], f32, tag="mu")
        nc.vector.tensor_reduce(out=mu[:], in_=x_sb[:], axis=Axis.X, op=Alu.add)
        nc.vector.tensor_scalar(mu[:], mu[:], 1.0 / (2 * H), None, Alu.mult)
        xc = pers.tile([B, 2 * H], f32, tag="xc")
        nc.vector.tensor_scalar(xc[:], x_sb[:], mu[:], None, Alu.subtract)
        sq = pers.tile([B, 2 * H], f32, tag="sq")
        var = pers.tile([B, 1], f32, tag="var")
        nc.scalar.activation(sq[:], xc[:], Act.Square, accum_out=var[:])
        sd = pers.tile([B, 1], f32, tag="sd")
        eps_t = pers.tile([B, 1], f32, tag="eps")
        nc.vector.memset(eps_t[:], LN_EPS)
        nc.scalar.activation(sd[:], var[:], Act.Sqrt, bias=eps_t[:], scale=1.0 / (2 * H))
        rs = pers.tile([B, 1], f32, tag="rs")
        nc.vector.reciprocal(out=rs[:], in_=sd[:])
        y = pers.tile([B, 2 * H], f32, tag="y")
        nc.vector.tensor_scalar(y[:], xc[:], rs[:], None, Alu.mult)
        if flags["ln_g"]:
            lng_sb = pers.tile([B, 2 * H], f32, tag="lng")
            nc.sync.dma_start(lng_sb[:], lng_d[:])
            nc.vector.tensor_tensor(out=y[:], in0=y[:], in1=lng_sb[:], op=Alu.mult)
        if flags["ln_b"]:
            lnb_sb = pers.tile([B, 2 * H], f32, tag="lnb")
            nc.sync.dma_start(lnb_sb[:], lnb_d[:])
            nc.vector.tensor_tensor(out=y[:], in0=y[:], in1=lnb_sb[:], op=Alu.add)
        h1_sb = pers.tile([B, 2 * H], f32, tag="h1")
        nc.scalar.activation(h1_sb[:], y[:], Act.Relu)

        # h1^T via PE transpose, then final matmul
        h1T = pers.tile([128, 2, B], f32, tag="h1T")
        pth = pp.tile([128, 2 * B], f32, tag="tp")
        for kc in range(2):
            nc.tensor.transpose(
                pth[:, kc * B:(kc + 1) * B],
                h1_sb[:, kc * 128:(kc + 1) * 128],
                ident[:B, :B],
            )
        nc.vector.tensor_copy(out=h1T[:].rearrange("p k b -> p (k b)"), in_=pth[:])

        po = pp.tile([B, H], f32, tag="gx")
        for kc in range(2):
            nc.tensor.matmul(
                po[:], lhsT=h1T[:, kc, :], rhs=w2_sb[:, kc, :],
                start=(kc == 0), stop=(kc == 1),
            )
        o_sb = pers.tile([B, H], f32, tag="osb")
        if flags["b2"]:
            b2_sb = pers.tile([B, H], f32, tag="b2")
            nc.sync.dma_start(b2_sb[:], b2_d[:])
            nc.vector.tensor_tensor(out=o_sb[:], in0=po[:], in1=b2_sb[:], op=Alu.add)
        else:
            nc.vector.tensor_copy(out=o_sb[:], in_=po[:])
        nc.sync.dma_start(out_d[:], o_sb[:])


def build_kernel(inputs):
    """Trace + compile; returns (nc, in_maps)."""
    from concourse import bacc, tile

    nodes = np.ascontiguousarray(
        np.asarray(inputs["nodes"], dtype=np.float32).reshape(N, 2 * H)
    )
    src = np.asarray(inputs["src"]).astype(np.int64)
    rel = np.asarray(inputs["rel"]).astype(np.int64)
    dst = np.asarray(inputs["dst"]).astype(np.int64)
    rel_W = np.ascontiguousarray(np.asarray(inputs["rel_W"], dtype=np.float32))
    rel_b = np.asarray(inputs["rel_b"], dtype=np.float32)
    gru_Wih = np.asarray(inputs["gru_Wih"], dtype=np.float32)
    gru_Whh = np.asarray(inputs["gru_Whh"], dtype=np.float32)
    gru_bih = np.asarray(inputs["gru_bih"], dtype=np.float32)
    gru_bhh = np.asarray(inputs["gru_bhh"], dtype=np.float32)
    mlp_W1 = np.ascontiguousarray(np.asarray(inputs["mlp_W1"], dtype=np.float32))
    mlp_b1 = np.asarray(inputs["mlp_b1"], dtype=np.float32)
    ln_g = np.asarray(inputs["ln_g"], dtype=np.float32)
    ln_b = np.asarray(inputs["ln_b"], dtype=np.float32)
    mlp_W2 = np.ascontiguousarray(np.asarray(inputs["mlp_W2"], dtype=np.float32))
    mlp_b2 = np.asarray(inputs["mlp_b2"], dtype=np.float32)
    tseq = np.ascontiguousarray(
        np.asarray(inputs["temporal_sequence"], dtype=np.float32).reshape(B * T, H)
    )

    # sigmoid-only GRU: tanh(x) = 2*sigmoid(2x) - 1, fold the 2x into n-gate
    # weights/biases (rows 2H:3H)
    wih2 = gru_Wih.copy()
    wih2[2 * H:] *= 2.0
    whh2 = gru_Whh.copy()
    whh2[2 * H:] *= 2.0
    bih2 = gru_bih.copy()
    bih2[2 * H:] *= 2.0
    bhh2 = gru_bhh.copy()
    bhh2[2 * H:] *= 2.0

    flags = {
        "rel_b": bool(np.any(rel_b != 0)),
        "bih": bool(np.any(bih2 != 0)),
        "bhh": bool(np.any(bhh2 != 0)),
        "ln_g": bool(np.any(ln_g != 1)),
        "ln_b": bool(np.any(ln_b != 0)),
        "b1": bool(np.any(mlp_b1 != 0)),
        "b2": bool(np.any(mlp_b2 != 0)),
    }

    nc = bacc.Bacc(
        "TRN2", target_bir_lowering=False, debug=False, num_devices=NCORES
    )
    with tile.TileContext(nc) as tc:
        _build(nc, tc, flags)
    nc.compile()

    in_maps = []
    for c in range(NCORES):
        lo, hi = c * EPC, (c + 1) * EPC
        s_pad = np.zeros(EPAD, np.int64)
        d_pad = np.zeros(EPAD, np.int64)
        r_pad = np.full(EPAD, R, np.int64)  # rel=R -> zero onehot row
        s_pad[:EPC] = src[lo:hi]
        d_pad[:EPC] = dst[lo:hi]
        r_pad[:EPC] = rel[lo:hi]
        m = {
            "nodes_flat": nodes,
            "sidx": _wrap_idx(s_pad),
            "didx": _wrap_idx(d_pad),
            "relv": np.ascontiguousarray(
                r_pad.reshape(TILES, 128).T.astype(np.float32)
            ),
            "relW": rel_W,
            "tseq": tseq,
            "wih": wih2,
            "whh": whh2,
            "w1": mlp_W1,
            "w2": mlp_W2,
        }
        if flags["rel_b"]:
            m["relb"] = rel_b
        if flags["bih"]:
            m["biht"] = np.ascontiguousarray(bih2.reshape(3, H).T)
        if flags["bhh"]:
            m["bhht"] = np.ascontiguousarray(bhh2.reshape(3, H).T)
        if flags["ln_g"]:
            m["lng"] = np.ascontiguousarray(np.broadcast_to(ln_g, (B, 2 * H)))
        if flags["ln_b"]:
            m["lnb"] = np.ascontiguousarray(np.broadcast_to(ln_b, (B, 2 * H)))
        if flags["b1"]:
            m["b1"] = np.ascontiguousarray(np.broadcast_to(mlp_b1, (B, 2 * H)))
        if flags["b2"]:
            m["b2"] = np.ascontiguousarray(np.broadcast_to(mlp_b2, (B, H)))
        in_maps.append(m)
    return nc, in_maps


def run(inputs, trace=False):
    from concourse import bass_utils

    nc, in_maps = build_kernel(inputs)
    res = bass_utils.run_bass_kernel_spmd(
        nc, in_maps, core_ids=list(range(NCORES)), trace=trace
    )
    return res


def kernel(**inputs):
    res = run(inputs, trace=False)
    return np.asarray(res.results[0]["out"], dtype=np.float32)


# revision 20
# speedup vs baseline: 1.7124x; 1.1571x over previous
"""NarrativeGraph GNN message-passing kernel for 8 Trainium2 NeuronCores.

Strategy (edge-sharded, per sharding hint):
  - E=50000 edges split 6250/core, padded to 6272 = 49 tiles of 128.
  - Per core: dma_gather src/dst node rows (1KB each) from HBM.
  - Relation-routed linear folded algebraically:
      sum_e edge_in[e] @ W[rel_e] = sum_r (S_r @ W_r),
      S_r = sum_{e: rel_e==r} concat(nodes[src_e], nodes[dst_e])
    computed as onehot(rel).T @ gathered_tile matmuls accumulating in PSUM.
  - 1KB AllReduce of the per-core partial graph state.
  - GRU (T=64 steps) + MLP replicated on every core, overlapping the gather.
"""

import sys

import numpy as np

_TRN = "/opt/trn_rl_repo"
if _TRN not in sys.path:
    sys.path.insert(0, _TRN)

H = 128
R = 8
N = 10000
E = 50000
B = 2
T = 64
NCORES = 8
EPC = E // NCORES          # 6250 edges per core
TILES = 49                 # ceil(6250/128)
EPAD = TILES * 128         # 6272
LN_EPS = 1e-5
GCHUNK = 7                 # gather chunk = 7 tiles = 896 edges
NCHUNKS = TILES // GCHUNK  # 7


def _wrap_idx(idx):
    """int16 wrapped layout for dma_gather: idx j at [j%16, j//16],
    replicated across the 8 gpsimd DSP stripes (partitions 16k..16k+15)."""
    w16 = idx.reshape(EPAD // 16, 16).T.astype(np.int16)
    return np.ascontiguousarray(np.tile(w16, (8, 1)))


def _build(nc, tc, flags):
    from concourse import bass
    import concourse.mybir as mybir

    f32 = mybir.dt.float32
    f32r = mybir.dt.float32r
    i16 = mybir.dt.int16
    Alu = mybir.AluOpType
    Act = mybir.ActivationFunctionType
    Axis = mybir.AxisListType

    # ---- DRAM I/O ----
    nodes_d = nc.dram_tensor("nodes_flat", [N, 2 * H], f32, kind="ExternalInput")
    sidx_d = nc.dram_tensor("sidx", [128, EPAD // 16], i16, kind="ExternalInput")
    didx_d = nc.dram_tensor("didx", [128, EPAD // 16], i16, kind="ExternalInput")
    relv_d = nc.dram_tensor("relv", [128, TILES], f32, kind="ExternalInput")
    relW_d = nc.dram_tensor("relW", [R, 2 * H, H], f32, kind="ExternalInput")
    tseq_d = nc.dram_tensor("tseq", [B * T, H], f32, kind="ExternalInput")
    wih_d = nc.dram_tensor("wih", [3 * H, H], f32, kind="ExternalInput")
    whh_d = nc.dram_tensor("whh", [3 * H, H], f32, kind="ExternalInput")
    w1_d = nc.dram_tensor("w1", [H, 2 * H], f32, kind="ExternalInput")
    w2_d = nc.dram_tensor("w2", [2 * H, H], f32, kind="ExternalInput")
    if flags["rel_b"]:
        relb_d = nc.dram_tensor("relb", [R, H], f32, kind="ExternalInput")
    if flags["bih"]:
        biht_d = nc.dram_tensor("biht", [H, 3], f32, kind="ExternalInput")
    if flags["bhh"]:
        bhht_d = nc.dram_tensor("bhht", [H, 3], f32, kind="ExternalInput")
    if flags["ln_g"]:
        lng_d = nc.dram_tensor("lng", [B, 2 * H], f32, kind="ExternalInput")
    if flags["ln_b"]:
        lnb_d = nc.dram_tensor("lnb", [B, 2 * H], f32, kind="ExternalInput")
    if flags["b1"]:
        b1_d = nc.dram_tensor("b1", [B, 2 * H], f32, kind="ExternalInput")
    if flags["b2"]:
        b2_d = nc.dram_tensor("b2", [B, H], f32, kind="ExternalInput")
    out_d = nc.dram_tensor("out", [B, H], f32, kind="ExternalOutput")

    from concourse.masks import make_identity
    from contextlib import ExitStack

    with ExitStack() as stk:
        pers = stk.enter_context(tc.tile_pool(name="pers", bufs=1))
        loop = stk.enter_context(tc.tile_pool(name="loop", bufs=3))
        pp = stk.enter_context(tc.tile_pool(name="pp", bufs=1, space="PSUM"))
        dram = stk.enter_context(tc.tile_pool(name="dram", bufs=1, space="DRAM"))

        # ---- persistent SBUF tiles ----
        ident = pers.tile([128, 128], f32, tag="ident")
        make_identity(nc, ident)

        gath = pers.tile([128, 2, TILES, 2 * H], f32r, tag="gath")
        oneh = pers.tile([128, TILES, R], f32, tag="oneh")
        oneh_r = pers.tile([128, TILES, R], f32r, tag="onehr")
        relv_sb = pers.tile([128, TILES], f32, tag="relv")
        iota_f = pers.tile([128, R], f32, tag="iota")
        sidx_sb = pers.tile([128, EPAD // 16], i16, tag="sidx")
        didx_sb = pers.tile([128, EPAD // 16], i16, tag="didx")
        relW_sb = pers.tile([128, R, 2, H], f32, tag="relW")
        ssb = pers.tile([R, 4 * H], f32, tag="ssb")
        st_sb = pers.tile([128, 2, B, R], f32, tag="st")
        xT = pers.tile([128, B, T], f32, tag="xT")
        wihT = pers.tile([128, 3, H], f32, tag="wihT")
        whhT = pers.tile([128, 3, H], f32, tag="whhT")
        gxT = pers.tile([128, 3, B, T], f32, tag="gxT")
        w1_sb = pers.tile([128, 2 * H], f32, tag="w1")
        w2_sb = pers.tile([128, 2, H], f32, tag="w2")
        gT_sb = pers.tile([128, B], f32, tag="gT")
        gfull = pers.tile([128, B], f32, tag="gfull")
        gsT = pers.tile([128, B], f32, tag="gsT")
        ones_sb = None
        if flags["rel_b"]:
            ones_sb = pers.tile([128, 1], f32, tag="ones")
            nc.vector.memset(ones_sb[:], 1.0)
            relb_sb = pers.tile([R, H], f32, tag="relb")
            nc.sync.dma_start(relb_sb[:], relb_d[:])
            cnt_sb = pers.tile([R, 1], f32, tag="cnt")

        # ---- input loads ----
        # DMA queue is serial (~500ns/op): load gather indices first (they
        # gate the 21us gather chain), then GRU weights; park relW/w1/w2
        # behind wait stamps since they're not needed until much later.
        nc.sync.dma_start(sidx_sb[:], sidx_d[:])
        nc.sync.dma_start(didx_sb[:], didx_d[:])
        nc.sync.dma_start(relv_sb[:], relv_d[:])
        with tc.tile_wait_until(0.008):
            for r in range(R):
                for kc in range(2):
                    nc.sync.dma_start(
                        relW_sb[:, r, kc, :], relW_d[r, kc * 128:(kc + 1) * 128, :]
                    )
        with tc.tile_wait_until(0.030):
            nc.sync.dma_start(w1_sb[:], w1_d[:])
            for kc in range(2):
                nc.sync.dma_start(w2_sb[:, kc, :], w2_d[kc * 128:(kc + 1) * 128, :])

        # iota 0..R-1 along free dim (8 tiny memsets; avoids gpsimd iota lib)
        for r in range(R):
            nc.vector.memset(iota_f[:, r:r + 1], float(r))

        # onehot for all tiles in one DVE op: oneh[p,t,r] = (relv[p,t] == r)
        nc.vector.tensor_tensor(
            out=oneh[:],
            in0=relv_sb[:].unsqueeze(2).to_broadcast([128, TILES, R]),
            in1=iota_f[:].unsqueeze(1).to_broadcast([128, TILES, R]),
            op=Alu.is_equal,
        )
        nc.vector.tensor_copy(
            out=oneh_r[:].rearrange("p t r -> p (t r)"),
            in_=oneh[:].rearrange("p t r -> p (t r)"),
        )

        # ---- GRU setup: transposes + gx precompute ----
        xbt = pers.tile([128, H], f32, tag="xbt")
        nc.sync.dma_start(xbt[:], tseq_d[:])
        pt = pp.tile([128, 128], f32, tag="tp")
        nc.tensor.transpose(pt[:], xbt[:], ident[:])
        nc.vector.tensor_copy(out=xT[:].rearrange("p b t -> p (b t)"), in_=pt[:])

        wall = pers.tile([128, 6, H], f32, tag="wall")
        nc.sync.dma_start(
            wall[:, 0:3, :], wih_d[:].rearrange("(g p) h -> p g h", g=3)
        )
        nc.sync.dma_start(
            wall[:, 3:6, :], whh_d[:].rearrange("(g p) h -> p g h", g=3)
        )
        for g in range(3):
            ptw = pp.tile([128, 128], f32, tag="tp")
            nc.tensor.transpose(ptw[:], wall[:, g, :], ident[:])
            nc.vector.tensor_copy(out=wihT[:, g, :], in_=ptw[:])
        for g in range(3):
            ptw = pp.tile([128, 128], f32, tag="tp")
            nc.tensor.transpose(ptw[:], wall[:, 3 + g, :], ident[:])
            nc.vector.tensor_copy(out=whhT[:, g, :], in_=ptw[:])

        pgx = pp.tile([128, 3, B * T], f32, tag="gx")
        for g in range(3):
            nc.tensor.matmul(
                pgx[:, g, :], lhsT=wihT[:, g, :], rhs=xT[:].rearrange("p b t -> p (b t)"),
                start=True, stop=True,
            )
        nc.vector.tensor_copy(
            out=gxT[:].rearrange("p g b t -> p (g b t)"),
            in_=pgx[:].rearrange("p g n -> p (g n)"),
        )
        # gxz = gxT with the n-gate zeroed: per-step PSUM inject for r/z
        # (gx_n enters via the ACT bias instead, since pgh[2] must be hn only)
        gxz = pers.tile([128, 3, B, T], f32, tag="gxz")
        nc.vector.tensor_copy(
            out=gxz[:, 0:2].rearrange("p g b t -> p (g b t)"),
            in_=gxT[:, 0:2].rearrange("p g b t -> p (g b t)"),
        )
        nc.vector.memset(gxz[:, 2].rearrange("p b t -> p (b t)"), 0.0)
        zstop = pers.tile([128, 3 * B], f32, tag="zstop")
        nc.vector.memset(zstop[:], 0.0)
        if flags["bih"]:
            biht_sb = pers.tile([128, 3], f32, tag="biht")
            nc.sync.dma_start(biht_sb[:], biht_d[:])
            nc.vector.tensor_tensor(
                out=gxT[:], in0=gxT[:],
                in1=biht_sb[:].unsqueeze(2).unsqueeze(3).to_broadcast([128, 3, B, T]),
                op=Alu.add,
            )
        if flags["bhh"]:
            bhht_sb = pers.tile([128, 3], f32, tag="bhht")
            nc.sync.dma_start(bhht_sb[:], bhht_d[:])
            nc.vector.tensor_tensor(
                out=gxT[:], in0=gxT[:],
                in1=bhht_sb[:].unsqueeze(2).unsqueeze(3).to_broadcast([128, 3, B, T]),
                op=Alu.add,
            )

        # ---- issue all gathers (SWDGE, overlapped with GRU) ----
        ecols = GCHUNK * 128 // 16  # idx cols per chunk
        for c in range(NCHUNKS):
            nc.gpsimd.dma_gather(
                gath[:, 0, c * GCHUNK:(c + 1) * GCHUNK, :],
                nodes_d[:].bitcast(f32r),
                sidx_sb[:, c * ecols:(c + 1) * ecols],
                GCHUNK * 128, GCHUNK * 128, 2 * H,
                queue_num=0,
            )
            nc.gpsimd.dma_gather(
                gath[:, 1, c * GCHUNK:(c + 1) * GCHUNK, :],
                nodes_d[:].bitcast(f32r),
                didx_sb[:, c * ecols:(c + 1) * ecols],
                GCHUNK * 128, GCHUNK * 128, 2 * H,
                queue_num=0,
            )

        # ---- S accumulation psum (lives across GRU loop) ----
        psum_s = pp.tile([R, 4 * H], f32, tag="S")

        def s_matmuls(tlo, thi):
            for t in range(tlo, thi):
                # rhs free order (b, s, col): matches W row layout [src|dst] per b
                rhs = gath[:, :, t, :].rearrange("p s (b c) -> p b s c", b=B)
                nc.tensor.matmul(
                    psum_s[:],
                    lhsT=oneh_r[:, t, :],
                    rhs=rhs,
                    start=(t == 0), stop=(t == TILES - 1),
                )
            if flags["rel_b"] and thi == TILES:
                pcnt = pp.tile([R, 1], f32, tag="cntp")
                for t in range(TILES):
                    nc.tensor.matmul(
                        pcnt[:], lhsT=oneh[:, t, :], rhs=ones_sb[:],
                        start=(t == 0), stop=(t == TILES - 1),
                    )
                nc.vector.tensor_copy(out=cnt_sb[:], in_=pcnt[:])

        # ---- GRU loop (critical chain: PE -> ACT -> DVE -> PE) ----
        # pgh[0:2] accumulates gx_{r,z} (identity inject) + Whh_{r,z} h in
        # PSUM so sigma reads it directly; n gate fused on ACT as
        # sigmoid(r*hn + gx_n) via scale/bias APs; blend uses
        # h_new = q*w2 + c with w2 = 2(1-z), c = z*(h_prev+1) - 1 where
        # w2/c are computed off the critical chain.
        h_prev = loop.tile([128, B], f32, tag="h", bufs=2)
        nc.vector.memset(h_prev[:], 0.0)
        hp1 = loop.tile([128, B], f32, tag="hp1", bufs=2)
        nc.vector.memset(hp1[:], 1.0)
        for t in range(T):
            pgh = pp.tile([128, 3, B], f32, tag="gh", bufs=2)
            # one zero region per PSUM bank: single start (inject, zeroes all
            # of pgh), unflagged accumulates, single stop (zero matmul)
            nc.tensor.matmul(
                pgh[:], lhsT=ident[:], rhs=gxz[:, :, :, t],
                start=True, stop=False,
            )
            for g in range(3):
                nc.tensor.matmul(
                    pgh[:, g, :], lhsT=whhT[:, g, :], rhs=h_prev[:],
                    start=False, stop=False,
                )
            nc.tensor.matmul(
                pgh[:].rearrange("p g b -> p (g b)"), lhsT=ident[:], rhs=zstop[:],
                start=False, stop=True,
            )
            rz = loop.tile([128, 2, B], f32, tag="rz", bufs=3)
            nc.scalar.activation(rz[:], pgh[:, 0:2, :], Act.Sigmoid)
            hnb = None
            if flags["bhh"]:
                hnb = loop.tile([128, B], f32, tag="hnb", bufs=3)
                nc.vector.tensor_tensor(
                    out=hnb[:], in0=pgh[:, 2, :],
                    in1=bhht_sb[:, 2:3].to_broadcast([128, B]),
                    op=Alu.add,
                )
            q_t = loop.tile([128, B], f32, tag="q", bufs=3)
            for b in range(B):
                src = hnb[:, b:b + 1] if hnb is not None else pgh[:, 2, b:b + 1]
                nc.scalar.activation(
                    q_t[:, b:b + 1], src, Act.Sigmoid,
                    bias=gxT[:, 2, b:b + 1, t], scale=rz[:, 0, b:b + 1],
                )
            w2t = loop.tile([128, B], f32, tag="w2t", bufs=3)
            nc.vector.tensor_scalar(w2t[:], rz[:, 1, :], -2.0, 2.0, Alu.mult, Alu.add)
            zh1 = loop.tile([128, B], f32, tag="zh1", bufs=3)
            nc.vector.tensor_tensor(out=zh1[:], in0=rz[:, 1, :], in1=hp1[:], op=Alu.mult)
            cbl = loop.tile([128, B], f32, tag="cbl", bufs=3)
            nc.vector.tensor_scalar(cbl[:], zh1[:], -1.0, None, Alu.add)
            t1 = loop.tile([128, B], f32, tag="t1", bufs=3)
            nc.vector.tensor_tensor(out=t1[:], in0=q_t[:], in1=w2t[:], op=Alu.mult)
            h_new = loop.tile([128, B], f32, tag="h", bufs=2)
            nc.vector.tensor_tensor(out=h_new[:], in0=t1[:], in1=cbl[:], op=Alu.add)
            hp1 = loop.tile([128, B], f32, tag="hp1", bufs=2)
            nc.vector.tensor_scalar(hp1[:], h_new[:], 1.0, None, Alu.add)
            h_prev = h_new

        # S-matmul chunks: wait_until stamps keep the scheduler from placing
        # them before their gathers land (it models gathers as ~instant),
        # so the GRU chain claims the PE from the start.
        for c in range(NCHUNKS):
            with tc.tile_wait_until(0.005 + 0.0031 * c):
                s_matmuls(c * GCHUNK, (c + 1) * GCHUNK)

        # ---- S -> G partial: transpose S then contract with rel_W ----
        nc.vector.tensor_copy(out=ssb[:], in_=psum_s[:])
        ptS = pp.tile([128, 2 * B * R], f32, tag="tp")
        for b in range(B):
            for kc in range(2):
                nc.tensor.transpose(
                    ptS[:, (kc * B + b) * R:(kc * B + b + 1) * R],
                    ssb[:, b * 2 * H + kc * H: b * 2 * H + (kc + 1) * H],
                    ident[:8, :8],
                )
        nc.vector.tensor_copy(
            out=st_sb[:].rearrange("p k b r -> p (k b r)"), in_=ptS[:]
        )

        pG = pp.tile([128, B], f32, tag="G")
        nmm = 2 * R + (1 if flags["rel_b"] else 0)
        i = 0
        for kc in range(2):
            for r in range(R):
                nc.tensor.matmul(
                    pG[:], lhsT=relW_sb[:, r, kc, :], rhs=st_sb[:, kc, :, r],
                    start=(i == 0), stop=(i == nmm - 1),
                )
                i += 1
        if flags["rel_b"]:
            nc.tensor.matmul(
                pG[:], lhsT=relb_sb[:], rhs=cnt_sb[:].to_broadcast([R, B]),
                start=False, stop=True,
            )
        nc.vector.tensor_copy(out=gT_sb[:], in_=pG[:])

        # ---- AllReduce the 1KB partial over the 8 cores ----
        cc_in = dram.tile([128, B], f32, tag="ccin")
        cc_out = dram.tile([128, B], f32, tag="ccout")
        nc.gpsimd.dma_start(cc_in[:], gT_sb[:])
        nc.gpsimd.collective_compute(
            "AllReduce",
            Alu.add,
            replica_groups=[list(range(NCORES))],
            ins=[cc_in[:]],
            outs=[cc_out[:]],
        )
        nc.gpsimd.dma_start(gfull[:], cc_out[:])

        # graph_state^T = G/E + h_last
        nc.vector.tensor_scalar(gsT[:], gfull[:], 1.0 / E, None, Alu.mult)
        nc.vector.tensor_tensor(out=gsT[:], in0=gsT[:], in1=h_prev[:], op=Alu.add)

        # ---- MLP + LayerNorm (row layout [B, .]) ----
        ph1 = pp.tile([B, 2 * H], f32, tag="mlp")
        nc.tensor.matmul(
            ph1[:], lhsT=gsT[:], rhs=w1_sb[:], start=True, stop=True,
        )
        x_sb = pers.tile([B, 2 * H], f32, tag="xsb")
        if flags["b1"]:
            b1_sb = pers.tile([B, 2 * H], f32, tag="b1")
            nc.sync.dma_start(b1_sb[:], b1_d[:])
            nc.vector.tensor_tensor(out=x_sb[:], in0=ph1[:], in1=b1_sb[:], op=Alu.add)
        else:
            nc.vector.tensor_copy(out=x_sb[:], in_=ph1[:])
        mu = pers.tile([B, 1], f32, tag="mu")
        nc.vector.tensor_reduce(out=mu[:], in_=x_sb[:], axis=Axis.X, op=Alu.add)
        nc.vector.tensor_scalar(mu[:], mu[:], 1.0 / (2 * H), None, Alu.mult)
        xc = pers.tile([B, 2 * H], f32, tag="xc")
        nc.vector.tensor_scalar(xc[:], x_sb[:], mu[:], None, Alu.subtract)
        sq = pers.tile([B, 2 * H], f32, tag="sq")
        var = pers.tile([B, 1], f32, tag="var")
        nc.scalar.activation(sq[:], xc[:], Act.Square, accum_out=var[:])
        sd = pers.tile([B, 1], f32, tag="sd")
        eps_t = pers.tile([B, 1], f32, tag="eps")
        nc.vector.memset(eps_t[:], LN_EPS)
        nc.scalar.activation(sd[:], var[:], Act.Sqrt, bias=eps_t[:], scale=1.0 / (2 * H))
        rs = pers.tile([B, 1], f32, tag="rs")
        nc.vector.reciprocal(out=rs[:], in_=sd[:])
        y = pers.tile([B, 2 * H], f32, tag="y")
        nc.vector.tensor_scalar(y[:], xc[:], rs[:], None, Alu.mult)
        if flags["ln_g"]:
            lng_sb = pers.tile([B, 2 * H], f32, tag="lng")
            nc.sync.dma_start(lng_sb[:], lng_d[:])
            nc.vector.tensor_tensor(out=y[:], in0=y[:], in1=lng_sb[:], op=Alu.mult)
        if flags["ln_b"]:
            lnb_sb = pers.tile([B, 2 * H], f32, tag="lnb")
            nc.sync.dma_start(lnb_sb[:], lnb_d[:])
            nc.vector.tensor_tensor(out=y[:], in0=y[:], in1=lnb_sb[:], op=Alu.add)
        h1_sb = pers.tile([B, 2 * H], f32, tag="h1")
        nc.scalar.activation(h1_sb[:], y[:], Act.Relu)

        # h1^T via PE transpose, then final matmul
        h1T = pers.tile([128, 2, B], f32, tag="h1T")
        pth = pp.tile([128, 2 * B], f32, tag="tp")
        for kc in range(2):
            nc.tensor.transpose(
                pth[:, kc * B:(kc + 1) * B],
                h1_sb[:, kc * 128:(kc + 1) * 128],
                ident[:B, :B],
            )
        nc.vector.tensor_copy(out=h1T[:].rearrange("p k b -> p (k b)"), in_=pth[:])

        po = pp.tile([B, H], f32, tag="gx")
        for kc in range(2):
            nc.tensor.matmul(
                po[:], lhsT=h1T[:, kc, :], rhs=w2_sb[:, kc, :],
                start=(kc == 0), stop=(kc == 1),
            )
        o_sb = pers.tile([B, H], f32, tag="osb")
        if flags["b2"]:
            b2_sb = pers.tile([B, H], f32, tag="b2")
            nc.sync.dma_start(b2_sb[:], b2_d[:])
            nc.vector.tensor_tensor(out=o_sb[:], in0=po[:], in1=b2_sb[:], op=Alu.add)
        else:
            nc.vector.tensor_copy(out=o_sb[:], in_=po[:])
        nc.sync.dma_start(out_d[:], o_sb[:])


def build_kernel(inputs):
    """Trace + compile; returns (nc, in_maps)."""
    from concourse import bacc, tile

    nodes = np.ascontiguousarray(
        np.asarray(inputs["nodes"], dtype=np.float32).reshape(N, 2 * H)
    )
    src = np.asarray(inputs["src"]).astype(np.int64)
    rel = np.asarray(inputs["rel"]).astype(np.int64)
    dst = np.asarray(inputs["dst"]).astype(np.int64)
    rel_W = np.ascontiguousarray(np.asarray(inputs["rel_W"], dtype=np.float32))
    rel_b = np.asarray(inputs["rel_b"], dtype=np.float32)
    gru_Wih = np.asarray(inputs["gru_Wih"], dtype=np.float32)
    gru_Whh = np.asarray(inputs["gru_Whh"], dtype=np.float32)
    gru_bih = np.asarray(inputs["gru_bih"], dtype=np.float32)
    gru_bhh = np.asarray(inputs["gru_bhh"], dtype=np.float32)
    mlp_W1 = np.ascontiguousarray(np.asarray(inputs["mlp_W1"], dtype=np.float32))
    mlp_b1 = np.asarray(inputs["mlp_b1"], dtype=np.float32)
    ln_g = np.asarray(inputs["ln_g"], dtype=np.float32)
    ln_b = np.asarray(inputs["ln_b"], dtype=np.float32)
    mlp_W2 = np.ascontiguousarray(np.asarray(inputs["mlp_W2"], dtype=np.float32))
    mlp_b2 = np.asarray(inputs["mlp_b2"], dtype=np.float32)
    tseq = np.ascontiguousarray(
        np.asarray(inputs["temporal_sequence"], dtype=np.float32).reshape(B * T, H)
    )

    # sigmoid-only GRU: tanh(x) = 2*sigmoid(2x) - 1, fold the 2x into n-gate
    # weights/biases (rows 2H:3H)
    wih2 = gru_Wih.copy()
    wih2[2 * H:] *= 2.0
    whh2 = gru_Whh.copy()
    whh2[2 * H:] *= 2.0
    bih2 = gru_bih.copy()
    bih2[2 * H:] *= 2.0
    bhh2 = gru_bhh.copy()
    bhh2[2 * H:] *= 2.0

    flags = {
        "rel_b": bool(np.any(rel_b != 0)),
        "bih": bool(np.any(bih2 != 0)),
        "bhh": bool(np.any(bhh2 != 0)),
        "ln_g": bool(np.any(ln_g != 1)),
        "ln_b": bool(np.any(ln_b != 0)),
        "b1": bool(np.any(mlp_b1 != 0)),
        "b2": bool(np.any(mlp_b2 != 0)),
    }

    nc = bacc.Bacc(
        "TRN2", target_bir_lowering=False, debug=False, num_devices=NCORES
    )
    with tile.TileContext(nc) as tc:
        _build(nc, tc, flags)
    nc.compile()

    in_maps = []
    for c in range(NCORES):
        lo, hi = c * EPC, (c + 1) * EPC
        s_pad = np.zeros(EPAD, np.int64)
        d_pad = np.zeros(EPAD, np.int64)
        r_pad = np.full(EPAD, R, np.int64)  # rel=R -> zero onehot row
        s_pad[:EPC] = src[lo:hi]
        d_pad[:EPC] = dst[lo:hi]
        r_pad[:EPC] = rel[lo:hi]
        m = {
            "nodes_flat": nodes,
            "sidx": _wrap_idx(s_pad),
            "didx": _wrap_idx(d_pad),
            "relv": np.ascontiguousarray(
                r_pad.reshape(TILES, 128).T.astype(np.float32)
            ),
            "relW": rel_W,
            "tseq": tseq,
            "wih": wih2,
            "whh": whh2,
            "w1": mlp_W1,
            "w2": mlp_W2,
        }
        if flags["rel_b"]:
            m["relb"] = rel_b
        if flags["bih"]:
            m["biht"] = np.ascontiguousarray(bih2.reshape(3, H).T)
        if flags["bhh"]:
            m["bhht"] = np.ascontiguousarray(bhh2.reshape(3, H).T)
        if flags["ln_g"]:
            m["lng"] = np.ascontiguousarray(np.broadcast_to(ln_g, (B, 2 * H)))
        if flags["ln_b"]:
            m["lnb"] = np.ascontiguousarray(np.broadcast_to(ln_b, (B, 2 * H)))
        if flags["b1"]:
            m["b1"] = np.ascontiguousarray(np.broadcast_to(mlp_b1, (B, 2 * H)))
        if flags["b2"]:
            m["b2"] = np.ascontiguousarray(np.broadcast_to(mlp_b2, (B, H)))
        in_maps.append(m)
    return nc, in_maps


def run(inputs, trace=False):
    from concourse import bass_utils

    nc, in_maps = build_kernel(inputs)
    res = bass_utils.run_bass_kernel_spmd(
        nc, in_maps, core_ids=list(range(NCORES)), trace=trace
    )
    return res


def kernel(**inputs):
    res = run(inputs, trace=False)
    return np.asarray(res.results[0]["out"], dtype=np.float32)


# revision 27
# speedup vs baseline: 1.7789x; 1.0388x over previous
"""NarrativeGraph GNN message-passing kernel for 8 Trainium2 NeuronCores.

Strategy (edge-sharded, per sharding hint):
  - E=50000 edges split 6250/core, padded to 6272 = 49 tiles of 128.
  - Per core: dma_gather src/dst node rows (1KB each) from HBM.
  - Relation-routed linear folded algebraically:
      sum_e edge_in[e] @ W[rel_e] = sum_r (S_r @ W_r),
      S_r = sum_{e: rel_e==r} concat(nodes[src_e], nodes[dst_e])
    computed as onehot(rel).T @ gathered_tile matmuls accumulating in PSUM.
  - 1KB AllReduce of the per-core partial graph state.
  - GRU (T=64 steps) + MLP replicated on every core, overlapping the gather.
"""

import sys

import numpy as np

_TRN = "/opt/trn_rl_repo"
if _TRN not in sys.path:
    sys.path.insert(0, _TRN)

H = 128
R = 8
N = 10000
E = 50000
B = 2
T = 64
NCORES = 8
EPC = E // NCORES          # 6250 edges per core
TILES = 49                 # ceil(6250/128)
EPAD = TILES * 128         # 6272
LN_EPS = 1e-5
GCHUNK = 7                 # gather chunk = 7 tiles = 896 edges
NCHUNKS = TILES // GCHUNK  # 7


def _wrap_idx(idx):
    """int16 wrapped layout for dma_gather: idx j at [j%16, j//16],
    replicated across the 8 gpsimd DSP stripes (partitions 16k..16k+15)."""
    w16 = idx.reshape(EPAD // 16, 16).T.astype(np.int16)
    return np.ascontiguousarray(np.tile(w16, (8, 1)))


def _build(nc, tc, flags):
    from concourse import bass
    import concourse.mybir as mybir

    f32 = mybir.dt.float32
    f32r = mybir.dt.float32r
    i16 = mybir.dt.int16
    Alu = mybir.AluOpType
    Act = mybir.ActivationFunctionType
    Axis = mybir.AxisListType

    # ---- DRAM I/O ----
    nodes_d = nc.dram_tensor("nodes_flat", [N, 2 * H], f32, kind="ExternalInput")
    sidx_d = nc.dram_tensor("sidx", [128, EPAD // 16], i16, kind="ExternalInput")
    didx_d = nc.dram_tensor("didx", [128, EPAD // 16], i16, kind="ExternalInput")
    relv_d = nc.dram_tensor("relv", [128, TILES], f32, kind="ExternalInput")
    relW_d = nc.dram_tensor("relW", [R, 2 * H, H], f32, kind="ExternalInput")
    tseq_d = nc.dram_tensor("tseq", [B * T, H], f32, kind="ExternalInput")
    wih_d = nc.dram_tensor("wih", [3 * H, H], f32, kind="ExternalInput")
    whh_d = nc.dram_tensor("whh", [3 * H, H], f32, kind="ExternalInput")
    w1_d = nc.dram_tensor("w1", [H, 2 * H], f32, kind="ExternalInput")
    w2_d = nc.dram_tensor("w2", [2 * H, H], f32, kind="ExternalInput")
    if flags["rel_b"]:
        relb_d = nc.dram_tensor("relb", [R, H], f32, kind="ExternalInput")
    if flags["bih"]:
        biht_d = nc.dram_tensor("biht", [H, 3], f32, kind="ExternalInput")
    if flags["bhh"]:
        bhht_d = nc.dram_tensor("bhht", [H, 3], f32, kind="ExternalInput")
    if flags["ln_g"]:
        lng_d = nc.dram_tensor("lng", [B, 2 * H], f32, kind="ExternalInput")
    if flags["ln_b"]:
        lnb_d = nc.dram_tensor("lnb", [B, 2 * H], f32, kind="ExternalInput")
    if flags["b1"]:
        b1_d = nc.dram_tensor("b1", [B, 2 * H], f32, kind="ExternalInput")
    if flags["b2"]:
        b2_d = nc.dram_tensor("b2", [B, H], f32, kind="ExternalInput")
    out_d = nc.dram_tensor("out", [B, H], f32, kind="ExternalOutput")

    from concourse.masks import make_identity
    from contextlib import ExitStack

    with ExitStack() as stk:
        pers = stk.enter_context(tc.tile_pool(name="pers", bufs=1))
        loop = stk.enter_context(tc.tile_pool(name="loop", bufs=3))
        pp = stk.enter_context(tc.tile_pool(name="pp", bufs=1, space="PSUM"))
        dram = stk.enter_context(tc.tile_pool(name="dram", bufs=1, space="DRAM"))

        # ---- persistent SBUF tiles ----
        ident = pers.tile([128, 128], f32, tag="ident")
        make_identity(nc, ident)

        gath = pers.tile([128, 2, TILES, 2 * H], f32r, tag="gath")
        oneh = pers.tile([128, TILES, R], f32, tag="oneh")
        oneh_r = pers.tile([128, TILES, R], f32r, tag="onehr")
        relv_sb = pers.tile([128, TILES], f32, tag="relv")
        iota_f = pers.tile([128, R], f32, tag="iota")
        sidx_sb = pers.tile([128, EPAD // 16], i16, tag="sidx")
        didx_sb = pers.tile([128, EPAD // 16], i16, tag="didx")
        relW_sb = pers.tile([128, R, 2, H], f32, tag="relW")
        ssb = pers.tile([R, 4 * H], f32, tag="ssb")
        st_sb = pers.tile([128, 2, B, R], f32, tag="st")
        xT = pers.tile([128, B, T], f32, tag="xT")
        wihT = pers.tile([128, 3, H], f32, tag="wihT")
        whhT = pers.tile([128, 3, H], f32, tag="whhT")
        gxT = pers.tile([128, 3, B, T], f32, tag="gxT")
        w1_sb = pers.tile([128, 2 * H], f32, tag="w1")
        w2_sb = pers.tile([128, 2, H], f32, tag="w2")
        gT_sb = pers.tile([128, B], f32, tag="gT")
        gfull = pers.tile([128, B], f32, tag="gfull")
        gsT = pers.tile([128, B], f32, tag="gsT")
        ones_sb = None
        if flags["rel_b"]:
            ones_sb = pers.tile([128, 1], f32, tag="ones")
            nc.vector.memset(ones_sb[:], 1.0)
            relb_sb = pers.tile([R, H], f32, tag="relb")
            nc.sync.dma_start(relb_sb[:], relb_d[:])
            cnt_sb = pers.tile([R, 1], f32, tag="cnt")

        # ---- input loads ----
        # DMA queue is serial (~500ns/op): load gather indices first (they
        # gate the 21us gather chain), then GRU weights; park relW/w1/w2
        # behind wait stamps since they're not needed until much later.
        nc.sync.dma_start(sidx_sb[:], sidx_d[:])
        nc.sync.dma_start(didx_sb[:], didx_d[:])
        nc.sync.dma_start(relv_sb[:], relv_d[:])
        with tc.tile_wait_until(0.008):
            for r in range(R):
                for kc in range(2):
                    nc.sync.dma_start(
                        relW_sb[:, r, kc, :], relW_d[r, kc * 128:(kc + 1) * 128, :]
                    )
        with tc.tile_wait_until(0.030):
            nc.sync.dma_start(w1_sb[:], w1_d[:])
            for kc in range(2):
                nc.sync.dma_start(w2_sb[:, kc, :], w2_d[kc * 128:(kc + 1) * 128, :])

        # ---- issue all gathers (SWDGE, overlapped with GRU) ----
        ecols = GCHUNK * 128 // 16  # idx cols per chunk
        for c in range(NCHUNKS):
            nc.gpsimd.dma_gather(
                gath[:, 0, c * GCHUNK:(c + 1) * GCHUNK, :],
                nodes_d[:].bitcast(f32r),
                sidx_sb[:, c * ecols:(c + 1) * ecols],
                GCHUNK * 128, GCHUNK * 128, 2 * H,
                queue_num=0,
            )
            nc.gpsimd.dma_gather(
                gath[:, 1, c * GCHUNK:(c + 1) * GCHUNK, :],
                nodes_d[:].bitcast(f32r),
                didx_sb[:, c * ecols:(c + 1) * ecols],
                GCHUNK * 128, GCHUNK * 128, 2 * H,
                queue_num=0,
            )

        # iota 0..R-1 along free dim (8 tiny memsets; avoids gpsimd iota lib)
        for r in range(R):
            nc.vector.memset(iota_f[:, r:r + 1], float(r))

        # onehot for all tiles in one DVE op: oneh[p,t,r] = (relv[p,t] == r)
        nc.vector.tensor_tensor(
            out=oneh[:],
            in0=relv_sb[:].unsqueeze(2).to_broadcast([128, TILES, R]),
            in1=iota_f[:].unsqueeze(1).to_broadcast([128, TILES, R]),
            op=Alu.is_equal,
        )
        nc.vector.tensor_copy(
            out=oneh_r[:].rearrange("p t r -> p (t r)"),
            in_=oneh[:].rearrange("p t r -> p (t r)"),
        )

        # ---- GRU setup: transposes + gx precompute ----
        xbt = pers.tile([128, H], f32, tag="xbt")
        nc.sync.dma_start(xbt[:], tseq_d[:])
        pt = pp.tile([128, 128], f32, tag="tp", bufs=2)
        nc.tensor.transpose(pt[:], xbt[:], ident[:])
        nc.vector.tensor_copy(out=xT[:].rearrange("p b t -> p (b t)"), in_=pt[:])

        wall = pers.tile([128, 6, H], f32, tag="wall")
        nc.sync.dma_start(
            wall[:, 0:3, :], wih_d[:].rearrange("(g p) h -> p g h", g=3)
        )
        nc.sync.dma_start(
            wall[:, 3:6, :], whh_d[:].rearrange("(g p) h -> p g h", g=3)
        )
        for g in range(3):
            ptw = pp.tile([128, 128], f32, tag="tp", bufs=2)
            nc.tensor.transpose(ptw[:], wall[:, g, :], ident[:])
            nc.vector.tensor_copy(out=wihT[:, g, :], in_=ptw[:])
        for g in range(3):
            ptw = pp.tile([128, 128], f32, tag="tp", bufs=2)
            nc.tensor.transpose(ptw[:], wall[:, 3 + g, :], ident[:])
            nc.vector.tensor_copy(out=whhT[:, g, :], in_=ptw[:])

        pgx = pp.tile([128, 3, B * T], f32, tag="gx")
        for g in range(3):
            nc.tensor.matmul(
                pgx[:, g, :], lhsT=wihT[:, g, :], rhs=xT[:].rearrange("p b t -> p (b t)"),
                start=True, stop=True,
            )
        nc.vector.tensor_copy(
            out=gxT[:].rearrange("p g b t -> p (g b t)"),
            in_=pgx[:].rearrange("p g n -> p (g n)"),
        )
        # gxz = gxT with the n-gate zeroed: per-step PSUM inject for r/z
        # (gx_n enters via the ACT bias instead, since pgh[2] must be hn only)
        gxz = pers.tile([128, 3, B, T], f32, tag="gxz")
        nc.vector.tensor_copy(
            out=gxz[:, 0:2].rearrange("p g b t -> p (g b t)"),
            in_=gxT[:, 0:2].rearrange("p g b t -> p (g b t)"),
        )
        nc.vector.memset(gxz[:, 2].rearrange("p b t -> p (b t)"), 0.0)
        zstop = pers.tile([128, 3 * B], f32, tag="zstop")
        nc.vector.memset(zstop[:], 0.0)
        if flags["bih"]:
            biht_sb = pers.tile([128, 3], f32, tag="biht")
            nc.sync.dma_start(biht_sb[:], biht_d[:])
            nc.vector.tensor_tensor(
                out=gxT[:], in0=gxT[:],
                in1=biht_sb[:].unsqueeze(2).unsqueeze(3).to_broadcast([128, 3, B, T]),
                op=Alu.add,
            )
        if flags["bhh"]:
            bhht_sb = pers.tile([128, 3], f32, tag="bhht")
            nc.sync.dma_start(bhht_sb[:], bhht_d[:])
            nc.vector.tensor_tensor(
                out=gxT[:], in0=gxT[:],
                in1=bhht_sb[:].unsqueeze(2).unsqueeze(3).to_broadcast([128, 3, B, T]),
                op=Alu.add,
            )

        # ---- S accumulation psum (lives across GRU loop) ----
        psum_s = pp.tile([R, 4 * H], f32, tag="S")

        def s_matmuls(tlo, thi):
            for t in range(tlo, thi):
                # rhs free order (b, s, col): matches W row layout [src|dst] per b
                rhs = gath[:, :, t, :].rearrange("p s (b c) -> p b s c", b=B)
                nc.tensor.matmul(
                    psum_s[:],
                    lhsT=oneh_r[:, t, :],
                    rhs=rhs,
                    start=(t == 0), stop=(t == TILES - 1),
                )
            if flags["rel_b"] and thi == TILES:
                pcnt = pp.tile([R, 1], f32, tag="cntp")
                for t in range(TILES):
                    nc.tensor.matmul(
                        pcnt[:], lhsT=oneh[:, t, :], rhs=ones_sb[:],
                        start=(t == 0), stop=(t == TILES - 1),
                    )
                nc.vector.tensor_copy(out=cnt_sb[:], in_=pcnt[:])

        # ---- GRU loop (critical chain: PE -> ACT -> DVE -> PE) ----
        # pgh[0:2] accumulates gx_{r,z} (identity inject) + Whh_{r,z} h in
        # PSUM so sigma reads it directly; n gate fused on ACT as
        # sigmoid(r*hn + gx_n) via scale/bias APs; blend uses
        # h_new = q*w2 + c with w2 = 2(1-z), c = z*(h_prev+1) - 1 where
        # w2/c are computed off the critical chain.
        h_prev = loop.tile([128, B], f32, tag="h", bufs=2)
        nc.vector.memset(h_prev[:], 0.0)
        hp1 = loop.tile([128, B], f32, tag="hp1", bufs=2)
        nc.vector.memset(hp1[:], 1.0)
        for t in range(T):
            pgh = pp.tile([128, 3, B], f32, tag="gh", bufs=2)
            # one zero region per PSUM bank: single start (inject, zeroes all
            # of pgh), unflagged accumulates, single stop (zero matmul)
            nc.tensor.matmul(
                pgh[:], lhsT=ident[:], rhs=gxz[:, :, :, t],
                start=True, stop=False,
            )
            for g in range(3):
                nc.tensor.matmul(
                    pgh[:, g, :], lhsT=whhT[:, g, :], rhs=h_prev[:],
                    start=False, stop=False,
                )
            nc.tensor.matmul(
                pgh[:].rearrange("p g b -> p (g b)"), lhsT=ident[:], rhs=zstop[:],
                start=False, stop=True,
            )
            rz = loop.tile([128, 2, B], f32, tag="rz", bufs=3)
            nc.scalar.activation(rz[:], pgh[:, 0:2, :], Act.Sigmoid)
            hnb = None
            if flags["bhh"]:
                hnb = loop.tile([128, B], f32, tag="hnb", bufs=3)
                nc.vector.tensor_tensor(
                    out=hnb[:], in0=pgh[:, 2, :],
                    in1=bhht_sb[:, 2:3].to_broadcast([128, B]),
                    op=Alu.add,
                )
            q_t = loop.tile([128, B], f32, tag="q", bufs=3)
            for b in range(B):
                src = hnb[:, b:b + 1] if hnb is not None else pgh[:, 2, b:b + 1]
                nc.scalar.activation(
                    q_t[:, b:b + 1], src, Act.Sigmoid,
                    bias=gxT[:, 2, b:b + 1, t], scale=rz[:, 0, b:b + 1],
                )
            w2t = loop.tile([128, B], f32, tag="w2t", bufs=3)
            nc.vector.tensor_scalar(w2t[:], rz[:, 1, :], -2.0, 2.0, Alu.mult, Alu.add)
            zh1 = loop.tile([128, B], f32, tag="zh1", bufs=3)
            nc.vector.tensor_tensor(out=zh1[:], in0=rz[:, 1, :], in1=hp1[:], op=Alu.mult)
            cbl = loop.tile([128, B], f32, tag="cbl", bufs=3)
            nc.vector.tensor_scalar(cbl[:], zh1[:], -1.0, None, Alu.add)
            t1 = loop.tile([128, B], f32, tag="t1", bufs=3)
            nc.vector.tensor_tensor(out=t1[:], in0=q_t[:], in1=w2t[:], op=Alu.mult)
            h_new = loop.tile([128, B], f32, tag="h", bufs=2)
            nc.vector.tensor_tensor(out=h_new[:], in0=t1[:], in1=cbl[:], op=Alu.add)
            hp1 = loop.tile([128, B], f32, tag="hp1", bufs=2)
            nc.vector.tensor_scalar(hp1[:], h_new[:], 1.0, None, Alu.add)
            h_prev = h_new

        # pre-trigger the Sqrt/Relu ACT table load (1283ns) while LN stats
        # run on DVE; the hp1 data-dep orders it after the last GRU sigmoid
        scrap = pers.tile([128, 1], f32, tag="scrap")
        nc.scalar.activation(scrap[:], hp1[:, 0:1], Act.Sqrt)

        # S-matmul chunks: wait_until stamps keep the scheduler from placing
        # them before their gathers land (it models gathers as ~instant),
        # so the GRU chain claims the PE from the start.
        for c in range(NCHUNKS):
            with tc.tile_wait_until(max(0.0062, 0.0046 + 0.003 * c)):
                s_matmuls(c * GCHUNK, (c + 1) * GCHUNK)

        # ---- S -> G partial: transpose S then contract with rel_W ----
        nc.vector.tensor_copy(out=ssb[:], in_=psum_s[:])
        ptS = pp.tile([128, 2 * B * R], f32, tag="tp", bufs=2)
        for b in range(B):
            for kc in range(2):
                nc.tensor.transpose(
                    ptS[:, (kc * B + b) * R:(kc * B + b + 1) * R],
                    ssb[:, b * 2 * H + kc * H: b * 2 * H + (kc + 1) * H],
                    ident[:8, :8],
                )
        nc.vector.tensor_copy(
            out=st_sb[:].rearrange("p k b r -> p (k b r)"), in_=ptS[:]
        )

        pG = pp.tile([128, B], f32, tag="G")
        nmm = 2 * R + (1 if flags["rel_b"] else 0)
        i = 0
        for kc in range(2):
            for r in range(R):
                nc.tensor.matmul(
                    pG[:], lhsT=relW_sb[:, r, kc, :], rhs=st_sb[:, kc, :, r],
                    start=(i == 0), stop=(i == nmm - 1),
                )
                i += 1
        if flags["rel_b"]:
            nc.tensor.matmul(
                pG[:], lhsT=relb_sb[:], rhs=cnt_sb[:].to_broadcast([R, B]),
                start=False, stop=True,
            )
        nc.vector.tensor_copy(out=gT_sb[:], in_=pG[:])

        # ---- AllReduce the 1KB partial over the 8 cores ----
        cc_in = dram.tile([128, B], f32, tag="ccin")
        cc_out = dram.tile([128, B], f32, tag="ccout")
        nc.gpsimd.dma_start(cc_in[:], gT_sb[:])
        nc.gpsimd.collective_compute(
            "AllReduce",
            Alu.add,
            replica_groups=[list(range(NCORES))],
            ins=[cc_in[:]],
            outs=[cc_out[:]],
        )
        nc.gpsimd.dma_start(gfull[:], cc_out[:])

        # graph_state^T = G/E + h_last
        nc.vector.tensor_scalar(gsT[:], gfull[:], 1.0 / E, None, Alu.mult)
        nc.vector.tensor_tensor(out=gsT[:], in0=gsT[:], in1=h_prev[:], op=Alu.add)

        # ---- MLP + LayerNorm (row layout [B, .]) ----
        ph1 = pp.tile([B, 2 * H], f32, tag="mlp")
        nc.tensor.matmul(
            ph1[:], lhsT=gsT[:], rhs=w1_sb[:], start=True, stop=True,
        )
        x_sb = pers.tile([B, 2 * H], f32, tag="xsb")
        if flags["b1"]:
            b1_sb = pers.tile([B, 2 * H], f32, tag="b1")
            nc.sync.dma_start(b1_sb[:], b1_d[:])
            nc.vector.tensor_tensor(out=x_sb[:], in0=ph1[:], in1=b1_sb[:], op=Alu.add)
        else:
            nc.vector.tensor_copy(out=x_sb[:], in_=ph1[:])
        mu = pers.tile([B, 1], f32, tag="mu")
        nc.vector.tensor_reduce(out=mu[:], in_=x_sb[:], axis=Axis.X, op=Alu.add)
        nc.vector.tensor_scalar(mu[:], mu[:], 1.0 / (2 * H), None, Alu.mult)
        xc = pers.tile([B, 2 * H], f32, tag="xc")
        nc.vector.tensor_scalar(xc[:], x_sb[:], mu[:], None, Alu.subtract)
        sq = pers.tile([B, 2 * H], f32, tag="sq")
        var = pers.tile([B, 1], f32, tag="var")
        nc.vector.tensor_tensor_reduce(
            out=sq[:], in0=xc[:], in1=xc[:], scale=1.0, scalar=0.0,
            op0=Alu.mult, op1=Alu.add, accum_out=var[:],
        )
        sd = pers.tile([B, 1], f32, tag="sd")
        eps_t = pers.tile([B, 1], f32, tag="eps")
        nc.vector.memset(eps_t[:], LN_EPS)
        nc.scalar.activation(sd[:], var[:], Act.Sqrt, bias=eps_t[:], scale=1.0 / (2 * H))
        rs = pers.tile([B, 1], f32, tag="rs")
        nc.vector.reciprocal(out=rs[:], in_=sd[:])
        y = pers.tile([B, 2 * H], f32, tag="y")
        nc.vector.tensor_scalar(y[:], xc[:], rs[:], None, Alu.mult)
        if flags["ln_g"]:
            lng_sb = pers.tile([B, 2 * H], f32, tag="lng")
            nc.sync.dma_start(lng_sb[:], lng_d[:])
            nc.vector.tensor_tensor(out=y[:], in0=y[:], in1=lng_sb[:], op=Alu.mult)
        if flags["ln_b"]:
            lnb_sb = pers.tile([B, 2 * H], f32, tag="lnb")
            nc.sync.dma_start(lnb_sb[:], lnb_d[:])
            nc.vector.tensor_tensor(out=y[:], in0=y[:], in1=lnb_sb[:], op=Alu.add)
        h1_sb = pers.tile([B, 2 * H], f32, tag="h1")
        nc.scalar.activation(h1_sb[:], y[:], Act.Relu)

        # h1^T via PE transpose, then final matmul
        h1T = pers.tile([128, 2, B], f32, tag="h1T")
        pth = pp.tile([128, 2 * B], f32, tag="tp", bufs=2)
        for kc in range(2):
            nc.tensor.transpose(
                pth[:, kc * B:(kc + 1) * B],
                h1_sb[:, kc * 128:(kc + 1) * 128],
                ident[:B, :B],
            )
        nc.vector.tensor_copy(out=h1T[:].rearrange("p k b -> p (k b)"), in_=pth[:])

        po = pp.tile([B, H], f32, tag="gx")
        for kc in range(2):
            nc.tensor.matmul(
                po[:], lhsT=h1T[:, kc, :], rhs=w2_sb[:, kc, :],
                start=(kc == 0), stop=(kc == 1),
            )
        o_sb = pers.tile([B, H], f32, tag="osb")
        if flags["b2"]:
            b2_sb = pers.tile([B, H], f32, tag="b2")
            nc.sync.dma_start(b2_sb[:], b2_d[:])
            nc.vector.tensor_tensor(out=o_sb[:], in0=po[:], in1=b2_sb[:], op=Alu.add)
        else:
            nc.vector.tensor_copy(out=o_sb[:], in_=po[:])
        nc.sync.dma_start(out_d[:], o_sb[:])


def build_kernel(inputs):
    """Trace + compile; returns (nc, in_maps)."""
    from concourse import bacc, tile

    nodes = np.ascontiguousarray(
        np.asarray(inputs["nodes"], dtype=np.float32).reshape(N, 2 * H)
    )
    src = np.asarray(inputs["src"]).astype(np.int64)
    rel = np.asarray(inputs["rel"]).astype(np.int64)
    dst = np.asarray(inputs["dst"]).astype(np.int64)
    rel_W = np.ascontiguousarray(np.asarray(inputs["rel_W"], dtype=np.float32))
    rel_b = np.asarray(inputs["rel_b"], dtype=np.float32)
    gru_Wih = np.asarray(inputs["gru_Wih"], dtype=np.float32)
    gru_Whh = np.asarray(inputs["gru_Whh"], dtype=np.float32)
    gru_bih = np.asarray(inputs["gru_bih"], dtype=np.float32)
    gru_bhh = np.asarray(inputs["gru_bhh"], dtype=np.float32)
    mlp_W1 = np.ascontiguousarray(np.asarray(inputs["mlp_W1"], dtype=np.float32))
    mlp_b1 = np.asarray(inputs["mlp_b1"], dtype=np.float32)
    ln_g = np.asarray(inputs["ln_g"], dtype=np.float32)
    ln_b = np.asarray(inputs["ln_b"], dtype=np.float32)
    mlp_W2 = np.ascontiguousarray(np.asarray(inputs["mlp_W2"], dtype=np.float32))
    mlp_b2 = np.asarray(inputs["mlp_b2"], dtype=np.float32)
    tseq = np.ascontiguousarray(
        np.asarray(inputs["temporal_sequence"], dtype=np.float32).reshape(B * T, H)
    )

    # sigmoid-only GRU: tanh(x) = 2*sigmoid(2x) - 1, fold the 2x into n-gate
    # weights/biases (rows 2H:3H)
    wih2 = gru_Wih.copy()
    wih2[2 * H:] *= 2.0
    whh2 = gru_Whh.copy()
    whh2[2 * H:] *= 2.0
    bih2 = gru_bih.copy()
    bih2[2 * H:] *= 2.0
    bhh2 = gru_bhh.copy()
    bhh2[2 * H:] *= 2.0

    flags = {
        "rel_b": bool(np.any(rel_b != 0)),
        "bih": bool(np.any(bih2 != 0)),
        "bhh": bool(np.any(bhh2 != 0)),
        "ln_g": bool(np.any(ln_g != 1)),
        "ln_b": bool(np.any(ln_b != 0)),
        "b1": bool(np.any(mlp_b1 != 0)),
        "b2": bool(np.any(mlp_b2 != 0)),
    }

    nc = bacc.Bacc(
        "TRN2", target_bir_lowering=False, debug=False, num_devices=NCORES
    )
    with tile.TileContext(nc) as tc:
        _build(nc, tc, flags)
    nc.compile()

    in_maps = []
    for c in range(NCORES):
        lo, hi = c * EPC, (c + 1) * EPC
        s_pad = np.zeros(EPAD, np.int64)
        d_pad = np.zeros(EPAD, np.int64)
        r_pad = np.full(EPAD, R, np.int64)  # rel=R -> zero onehot row
        s_pad[:EPC] = src[lo:hi]
        d_pad[:EPC] = dst[lo:hi]
        r_pad[:EPC] = rel[lo:hi]
        m = {
            "nodes_flat": nodes,
            "sidx": _wrap_idx(s_pad),
            "didx": _wrap_idx(d_pad),
            "relv": np.ascontiguousarray(
                r_pad.reshape(TILES, 128).T.astype(np.float32)
            ),
            "relW": rel_W,
            "tseq": tseq,
            "wih": wih2,
            "whh": whh2,
            "w1": mlp_W1,
            "w2": mlp_W2,
        }
        if flags["rel_b"]:
            m["relb"] = rel_b
        if flags["bih"]:
            m["biht"] = np.ascontiguousarray(bih2.reshape(3, H).T)
        if flags["bhh"]:
            m["bhht"] = np.ascontiguousarray(bhh2.reshape(3, H).T)
        if flags["ln_g"]:
            m["lng"] = np.ascontiguousarray(np.broadcast_to(ln_g, (B, 2 * H)))
        if flags["ln_b"]:
            m["lnb"] = np.ascontiguousarray(np.broadcast_to(ln_b, (B, 2 * H)))
        if flags["b1"]:
            m["b1"] = np.ascontiguousarray(np.broadcast_to(mlp_b1, (B, 2 * H)))
        if flags["b2"]:
            m["b2"] = np.ascontiguousarray(np.broadcast_to(mlp_b2, (B, H)))
        in_maps.append(m)
    return nc, in_maps


def run(inputs, trace=False):
    from concourse import bass_utils

    nc, in_maps = build_kernel(inputs)
    res = bass_utils.run_bass_kernel_spmd(
        nc, in_maps, core_ids=list(range(NCORES)), trace=trace
    )
    return res


def kernel(**inputs):
    res = run(inputs, trace=False)
    return np.asarray(res.results[0]["out"], dtype=np.float32)


# revision 30
# speedup vs baseline: 2.0307x; 1.1416x over previous
"""NarrativeGraph GNN message-passing kernel for 8 Trainium2 NeuronCores.

Strategy (edge-sharded, per sharding hint):
  - E=50000 edges split 6250/core, padded to 6272 = 49 tiles of 128.
  - Per core: dma_gather src/dst node rows (1KB each) from HBM.
  - Relation-routed linear folded algebraically:
      sum_e edge_in[e] @ W[rel_e] = sum_r (S_r @ W_r),
      S_r = sum_{e: rel_e==r} concat(nodes[src_e], nodes[dst_e])
    computed as onehot(rel).T @ gathered_tile matmuls accumulating in PSUM.
  - 1KB AllReduce of the per-core partial graph state.
  - GRU (T=64 steps) + MLP replicated on every core, overlapping the gather.
"""

import sys

import numpy as np

_TRN = "/opt/trn_rl_repo"
if _TRN not in sys.path:
    sys.path.insert(0, _TRN)

H = 128
R = 8
N = 10000
E = 50000
B = 2
T = 64
NCORES = 8
EPC = E // NCORES          # 6250 edges per core
TILES = 49                 # ceil(6250/128)
EPAD = TILES * 128         # 6272
LN_EPS = 1e-5
GCHUNK = 7                 # gather chunk = 7 tiles = 896 edges
NCHUNKS = TILES // GCHUNK  # 7


def _wrap_idx(idx):
    """int16 wrapped layout for dma_gather: idx j at [j%16, j//16],
    replicated across the 8 gpsimd DSP stripes (partitions 16k..16k+15)."""
    w16 = idx.reshape(EPAD // 16, 16).T.astype(np.int16)
    return np.ascontiguousarray(np.tile(w16, (8, 1)))


def _build(nc, tc, flags):
    from concourse import bass
    import concourse.mybir as mybir

    f32 = mybir.dt.float32
    f32r = mybir.dt.float32r
    i16 = mybir.dt.int16
    Alu = mybir.AluOpType
    Act = mybir.ActivationFunctionType
    Axis = mybir.AxisListType

    # ---- DRAM I/O ----
    nodes_d = nc.dram_tensor("nodes_flat", [N, 2 * H], f32, kind="ExternalInput")
    sidx_d = nc.dram_tensor("sidx", [128, EPAD // 16], i16, kind="ExternalInput")
    didx_d = nc.dram_tensor("didx", [128, EPAD // 16], i16, kind="ExternalInput")
    relv_d = nc.dram_tensor("relv", [128, TILES], f32, kind="ExternalInput")
    relW_d = nc.dram_tensor("relW", [R, 2 * H, H], f32, kind="ExternalInput")
    tseq_d = nc.dram_tensor("tseq", [B * T, H], f32, kind="ExternalInput")
    wih_d = nc.dram_tensor("wih", [3 * H, H], f32, kind="ExternalInput")
    whh_d = nc.dram_tensor("whh", [3 * H, H], f32, kind="ExternalInput")
    w1_d = nc.dram_tensor("w1", [H, 2 * H], f32, kind="ExternalInput")
    w2_d = nc.dram_tensor("w2", [2 * H, H], f32, kind="ExternalInput")
    if flags["rel_b"]:
        relb_d = nc.dram_tensor("relb", [R, H], f32, kind="ExternalInput")
    if flags["bih"]:
        biht_d = nc.dram_tensor("biht", [H, 3], f32, kind="ExternalInput")
    if flags["bhh"]:
        bhht_d = nc.dram_tensor("bhht", [H, 3], f32, kind="ExternalInput")
    if flags["ln_g"]:
        lng_d = nc.dram_tensor("lng", [B, 2 * H], f32, kind="ExternalInput")
    if flags["ln_b"]:
        lnb_d = nc.dram_tensor("lnb", [B, 2 * H], f32, kind="ExternalInput")
    if flags["b1"]:
        b1_d = nc.dram_tensor("b1", [B, 2 * H], f32, kind="ExternalInput")
    if flags["b2"]:
        b2_d = nc.dram_tensor("b2", [B, H], f32, kind="ExternalInput")
    out_d = nc.dram_tensor("out", [B, H], f32, kind="ExternalOutput")

    from concourse.masks import make_identity
    from contextlib import ExitStack

    with ExitStack() as stk:
        pers = stk.enter_context(tc.tile_pool(name="pers", bufs=1))
        loop = stk.enter_context(tc.tile_pool(name="loop", bufs=3))
        pp = stk.enter_context(tc.tile_pool(name="pp", bufs=1, space="PSUM"))
        dram = stk.enter_context(tc.tile_pool(name="dram", bufs=1, space="DRAM"))

        # ---- persistent SBUF tiles ----
        ident = pers.tile([128, 128], f32, tag="ident")
        make_identity(nc, ident)

        gath = pers.tile([128, 2, TILES, 2 * H], f32r, tag="gath")
        oneh = pers.tile([128, TILES, R], f32, tag="oneh")
        oneh_r = pers.tile([128, TILES, R], f32r, tag="onehr")
        relv_sb = pers.tile([128, TILES], f32, tag="relv")
        iota_f = pers.tile([128, R], f32, tag="iota")
        sidx_sb = pers.tile([128, EPAD // 16], i16, tag="sidx")
        didx_sb = pers.tile([128, EPAD // 16], i16, tag="didx")
        relW_sb = pers.tile([128, R, 2, H], f32, tag="relW")
        ssb = pers.tile([R, 4 * H], f32, tag="ssb")
        st_sb = pers.tile([128, 2, B, R], f32, tag="st")
        xT = pers.tile([128, B, T], f32, tag="xT")
        wihT = pers.tile([128, 3, H], f32, tag="wihT")
        whhT = pers.tile([128, 3, H], f32, tag="whhT")
        gxT = pers.tile([128, 3, B, T], f32, tag="gxT")
        w1_sb = pers.tile([128, 2 * H], f32, tag="w1")
        w2_sb = pers.tile([128, 2, H], f32, tag="w2")
        gT_sb = pers.tile([128, B], f32, tag="gT")
        gfull = pers.tile([128, B], f32, tag="gfull")
        gsT = pers.tile([128, B], f32, tag="gsT")
        ones_sb = None
        if flags["rel_b"]:
            ones_sb = pers.tile([128, 1], f32, tag="ones")
            nc.vector.memset(ones_sb[:], 1.0)
            relb_sb = pers.tile([R, H], f32, tag="relb")
            nc.sync.dma_start(relb_sb[:], relb_d[:])
            cnt_sb = pers.tile([R, 1], f32, tag="cnt")

        # ---- input loads ----
        # DMA queue is serial (~500ns/op): load gather indices first (they
        # gate the 21us gather chain), then GRU weights; park relW/w1/w2
        # behind wait stamps since they're not needed until much later.
        nc.sync.dma_start(sidx_sb[:], sidx_d[:])
        nc.sync.dma_start(didx_sb[:], didx_d[:])
        nc.sync.dma_start(relv_sb[:], relv_d[:])
        with tc.tile_wait_until(0.008):
            for r in range(R):
                for kc in range(2):
                    nc.sync.dma_start(
                        relW_sb[:, r, kc, :], relW_d[r, kc * 128:(kc + 1) * 128, :]
                    )
        with tc.tile_wait_until(0.030):
            nc.sync.dma_start(w1_sb[:], w1_d[:])
            for kc in range(2):
                nc.sync.dma_start(w2_sb[:, kc, :], w2_d[kc * 128:(kc + 1) * 128, :])

        # ---- issue all gathers (SWDGE, overlapped with GRU) ----
        ecols = GCHUNK * 128 // 16  # idx cols per chunk
        for c in range(NCHUNKS):
            nc.gpsimd.dma_gather(
                gath[:, 0, c * GCHUNK:(c + 1) * GCHUNK, :],
                nodes_d[:].bitcast(f32r),
                sidx_sb[:, c * ecols:(c + 1) * ecols],
                GCHUNK * 128, GCHUNK * 128, 2 * H,
                queue_num=0,
            )
            nc.gpsimd.dma_gather(
                gath[:, 1, c * GCHUNK:(c + 1) * GCHUNK, :],
                nodes_d[:].bitcast(f32r),
                didx_sb[:, c * ecols:(c + 1) * ecols],
                GCHUNK * 128, GCHUNK * 128, 2 * H,
                queue_num=0,
            )

        # iota 0..R-1 along free dim (8 tiny memsets; avoids gpsimd iota lib)
        for r in range(R):
            nc.vector.memset(iota_f[:, r:r + 1], float(r))

        # onehot for all tiles in one DVE op: oneh[p,t,r] = (relv[p,t] == r)
        nc.vector.tensor_tensor(
            out=oneh[:],
            in0=relv_sb[:].unsqueeze(2).to_broadcast([128, TILES, R]),
            in1=iota_f[:].unsqueeze(1).to_broadcast([128, TILES, R]),
            op=Alu.is_equal,
        )
        nc.vector.tensor_copy(
            out=oneh_r[:].rearrange("p t r -> p (t r)"),
            in_=oneh[:].rearrange("p t r -> p (t r)"),
        )

        # ---- GRU setup: transposes + gx precompute ----
        xbt = pers.tile([128, H], f32, tag="xbt")
        nc.sync.dma_start(xbt[:], tseq_d[:])
        pt = pp.tile([128, 128], f32, tag="tp", bufs=2)
        nc.tensor.transpose(pt[:], xbt[:], ident[:])
        nc.vector.tensor_copy(out=xT[:].rearrange("p b t -> p (b t)"), in_=pt[:])

        wall = pers.tile([128, 6, H], f32, tag="wall")
        nc.sync.dma_start(
            wall[:, 0:3, :], wih_d[:].rearrange("(g p) h -> p g h", g=3)
        )
        nc.sync.dma_start(
            wall[:, 3:6, :], whh_d[:].rearrange("(g p) h -> p g h", g=3)
        )
        for g in range(3):
            ptw = pp.tile([128, 128], f32, tag="tp", bufs=2)
            nc.tensor.transpose(ptw[:], wall[:, g, :], ident[:])
            nc.vector.tensor_copy(out=wihT[:, g, :], in_=ptw[:])
        for g in range(3):
            ptw = pp.tile([128, 128], f32, tag="tp", bufs=2)
            nc.tensor.transpose(ptw[:], wall[:, 3 + g, :], ident[:])
            nc.vector.tensor_copy(out=whhT[:, g, :], in_=ptw[:])

        pgx = pp.tile([128, 3, B * T], f32, tag="gx")
        for g in range(3):
            nc.tensor.matmul(
                pgx[:, g, :], lhsT=wihT[:, g, :], rhs=xT[:].rearrange("p b t -> p (b t)"),
                start=True, stop=True,
            )
        nc.vector.tensor_copy(
            out=gxT[:].rearrange("p g b t -> p (g b t)"),
            in_=pgx[:].rearrange("p g n -> p (g n)"),
        )
        # gxz = gxT with the n-gate zeroed: per-step PSUM inject for r/z
        # (gx_n enters via the ACT bias instead, since pgh[2] must be hn only)
        gxz = pers.tile([128, 3, B, T], f32, tag="gxz")
        nc.vector.tensor_copy(
            out=gxz[:, 0:2].rearrange("p g b t -> p (g b t)"),
            in_=gxT[:, 0:2].rearrange("p g b t -> p (g b t)"),
        )
        nc.vector.memset(gxz[:, 2].rearrange("p b t -> p (b t)"), 0.0)
        zstop = pers.tile([128, 3 * B], f32, tag="zstop")
        nc.vector.memset(zstop[:], 0.0)
        if flags["bih"]:
            biht_sb = pers.tile([128, 3], f32, tag="biht")
            nc.sync.dma_start(biht_sb[:], biht_d[:])
            nc.vector.tensor_tensor(
                out=gxT[:], in0=gxT[:],
                in1=biht_sb[:].unsqueeze(2).unsqueeze(3).to_broadcast([128, 3, B, T]),
                op=Alu.add,
            )
        if flags["bhh"]:
            bhht_sb = pers.tile([128, 3], f32, tag="bhht")
            nc.sync.dma_start(bhht_sb[:], bhht_d[:])
            nc.vector.tensor_tensor(
                out=gxT[:], in0=gxT[:],
                in1=bhht_sb[:].unsqueeze(2).unsqueeze(3).to_broadcast([128, 3, B, T]),
                op=Alu.add,
            )

        # ---- S accumulation psum (lives across GRU loop) ----
        psum_s = pp.tile([R, 4 * H], f32, tag="S")

        def s_matmuls(tlo, thi):
            for t in range(tlo, thi):
                # rhs free order (b, s, col): matches W row layout [src|dst] per b
                rhs = gath[:, :, t, :].rearrange("p s (b c) -> p b s c", b=B)
                nc.tensor.matmul(
                    psum_s[:],
                    lhsT=oneh_r[:, t, :],
                    rhs=rhs,
                    start=(t == 0), stop=(t == TILES - 1),
                )
            if flags["rel_b"] and thi == TILES:
                pcnt = pp.tile([R, 1], f32, tag="cntp")
                for t in range(TILES):
                    nc.tensor.matmul(
                        pcnt[:], lhsT=oneh[:, t, :], rhs=ones_sb[:],
                        start=(t == 0), stop=(t == TILES - 1),
                    )
                nc.vector.tensor_copy(out=cnt_sb[:], in_=pcnt[:])

        # ---- GRU loop (critical chain: PE -> ACT -> DVE -> PE) ----
        # pgh[0:2] accumulates gx_{r,z} (identity inject) + Whh_{r,z} h in
        # PSUM so sigma reads it directly; n gate fused on ACT as
        # sigmoid(r*hn + gx_n) via scale/bias APs; blend uses
        # h_new = q*w2 + c with w2 = 2(1-z), c = z*(h_prev+1) - 1 where
        # w2/c are computed off the critical chain.
        cst = pers.tile([128, 3], f32, tag="cst")
        nc.vector.memset(cst[:, 0:1], 2.0)
        nc.vector.memset(cst[:, 1:2], -2.0)
        nc.vector.memset(cst[:, 2:3], -1.0)
        h_prev = loop.tile([128, B], f32, tag="h", bufs=2)
        nc.vector.memset(h_prev[:], 0.0)
        hp1 = loop.tile([128, B], f32, tag="hp1", bufs=2)
        nc.vector.memset(hp1[:], 1.0)
        for t in range(T):
            pgh = pp.tile([128, 3, B], f32, tag="gh", bufs=2)
            # one zero region per PSUM bank: single start (inject, zeroes all
            # of pgh), unflagged accumulates, single stop (zero matmul)
            nc.tensor.matmul(
                pgh[:], lhsT=ident[:], rhs=gxz[:, :, :, t],
                start=True, stop=False,
            )
            for g in range(3):
                nc.tensor.matmul(
                    pgh[:, g, :], lhsT=whhT[:, g, :], rhs=h_prev[:],
                    start=False, stop=False,
                )
            nc.tensor.matmul(
                pgh[:].rearrange("p g b -> p (g b)"), lhsT=ident[:], rhs=zstop[:],
                start=False, stop=True,
            )
            rz = loop.tile([128, 2, B], f32, tag="rz", bufs=3)
            nc.scalar.activation(rz[:], pgh[:, 0:2, :], Act.Sigmoid)
            hnb = None
            if flags["bhh"]:
                hnb = loop.tile([128, B], f32, tag="hnb", bufs=3)
                nc.vector.tensor_tensor(
                    out=hnb[:], in0=pgh[:, 2, :],
                    in1=bhht_sb[:, 2:3].to_broadcast([128, B]),
                    op=Alu.add,
                )
            q_t = loop.tile([128, B], f32, tag="q", bufs=3)
            for b in range(B):
                src = hnb[:, b:b + 1] if hnb is not None else pgh[:, 2, b:b + 1]
                nc.scalar.activation(
                    q_t[:, b:b + 1], src, Act.Sigmoid,
                    bias=gxT[:, 2, b:b + 1, t], scale=rz[:, 0, b:b + 1],
                )
            # blend entirely on ACT as per-column func(in*scale+bias) ops so
            # the whole step chain is PE -> sigma(rz) -> in-engine ACT -> PE
            w2t = loop.tile([128, B], f32, tag="w2t", bufs=3)
            cbl = loop.tile([128, B], f32, tag="cbl", bufs=3)
            h_new = loop.tile([128, B], f32, tag="h", bufs=2)
            hp1n = loop.tile([128, B], f32, tag="hp1", bufs=2)
            for b in range(B):
                nc.scalar.activation(
                    w2t[:, b:b + 1], rz[:, 1, b:b + 1], Act.Identity,
                    scale=cst[:, 1:2], bias=cst[:, 0:1])
                nc.scalar.activation(
                    cbl[:, b:b + 1], hp1[:, b:b + 1], Act.Identity,
                    scale=rz[:, 1, b:b + 1], bias=cst[:, 2:3])
                nc.scalar.activation(
                    h_new[:, b:b + 1], w2t[:, b:b + 1], Act.Identity,
                    scale=q_t[:, b:b + 1], bias=cbl[:, b:b + 1])
                nc.scalar.activation(
                    hp1n[:, b:b + 1], h_new[:, b:b + 1], Act.Identity,
                    scale=1.0, bias=1.0)
            hp1 = hp1n
            h_prev = h_new

        # pre-trigger the Sqrt/Relu ACT table load (1283ns) while LN stats
        # run on DVE; the hp1 data-dep orders it after the last GRU sigmoid
        scrap = pers.tile([128, 1], f32, tag="scrap")
        nc.scalar.activation(scrap[:], hp1[:, 0:1], Act.Sqrt)

        # S-matmul chunks: wait_until stamps keep the scheduler from placing
        # them before their gathers land (it models gathers as ~instant),
        # so the GRU chain claims the PE from the start.
        for c in range(NCHUNKS):
            with tc.tile_wait_until(max(0.0062, 0.0046 + 0.003 * c)):
                s_matmuls(c * GCHUNK, (c + 1) * GCHUNK)

        # ---- S -> G partial: transpose S then contract with rel_W ----
        nc.vector.tensor_copy(out=ssb[:], in_=psum_s[:])
        ptS = pp.tile([128, 2 * B * R], f32, tag="tp", bufs=2)
        for b in range(B):
            for kc in range(2):
                nc.tensor.transpose(
                    ptS[:, (kc * B + b) * R:(kc * B + b + 1) * R],
                    ssb[:, b * 2 * H + kc * H: b * 2 * H + (kc + 1) * H],
                    ident[:8, :8],
                )
        nc.vector.tensor_copy(
            out=st_sb[:].rearrange("p k b r -> p (k b r)"), in_=ptS[:]
        )

        pG = pp.tile([128, B], f32, tag="G")
        nmm = 2 * R + (1 if flags["rel_b"] else 0)
        i = 0
        for kc in range(2):
            for r in range(R):
                nc.tensor.matmul(
                    pG[:], lhsT=relW_sb[:, r, kc, :], rhs=st_sb[:, kc, :, r],
                    start=(i == 0), stop=(i == nmm - 1),
                )
                i += 1
        if flags["rel_b"]:
            nc.tensor.matmul(
                pG[:], lhsT=relb_sb[:], rhs=cnt_sb[:].to_broadcast([R, B]),
                start=False, stop=True,
            )
        nc.vector.tensor_copy(out=gT_sb[:], in_=pG[:])

        # ---- AllReduce the 1KB partial over the 8 cores ----
        cc_in = dram.tile([128, B], f32, tag="ccin")
        cc_out = dram.tile([128, B], f32, tag="ccout")
        nc.gpsimd.dma_start(cc_in[:], gT_sb[:])
        nc.gpsimd.collective_compute(
            "AllReduce",
            Alu.add,
            replica_groups=[list(range(NCORES))],
            ins=[cc_in[:]],
            outs=[cc_out[:]],
        )
        nc.gpsimd.dma_start(gfull[:], cc_out[:])

        # graph_state^T = G/E + h_last
        nc.vector.tensor_scalar(gsT[:], gfull[:], 1.0 / E, None, Alu.mult)
        nc.vector.tensor_tensor(out=gsT[:], in0=gsT[:], in1=h_prev[:], op=Alu.add)

        # ---- MLP + LayerNorm (row layout [B, .]) ----
        ph1 = pp.tile([B, 2 * H], f32, tag="mlp")
        nc.tensor.matmul(
            ph1[:], lhsT=gsT[:], rhs=w1_sb[:], start=True, stop=True,
        )
        x_sb = pers.tile([B, 2 * H], f32, tag="xsb")
        if flags["b1"]:
            b1_sb = pers.tile([B, 2 * H], f32, tag="b1")
            nc.sync.dma_start(b1_sb[:], b1_d[:])
            nc.vector.tensor_tensor(out=x_sb[:], in0=ph1[:], in1=b1_sb[:], op=Alu.add)
        else:
            nc.vector.tensor_copy(out=x_sb[:], in_=ph1[:])
        mu = pers.tile([B, 1], f32, tag="mu")
        nc.vector.tensor_reduce(out=mu[:], in_=x_sb[:], axis=Axis.X, op=Alu.add)
        nc.vector.tensor_scalar(mu[:], mu[:], 1.0 / (2 * H), None, Alu.mult)
        xc = pers.tile([B, 2 * H], f32, tag="xc")
        nc.vector.tensor_scalar(xc[:], x_sb[:], mu[:], None, Alu.subtract)
        sq = pers.tile([B, 2 * H], f32, tag="sq")
        var = pers.tile([B, 1], f32, tag="var")
        nc.vector.tensor_tensor_reduce(
            out=sq[:], in0=xc[:], in1=xc[:], scale=1.0, scalar=0.0,
            op0=Alu.mult, op1=Alu.add, accum_out=var[:],
        )
        sd = pers.tile([B, 1], f32, tag="sd")
        eps_t = pers.tile([B, 1], f32, tag="eps")
        nc.vector.memset(eps_t[:], LN_EPS)
        nc.scalar.activation(sd[:], var[:], Act.Sqrt, bias=eps_t[:], scale=1.0 / (2 * H))
        rs = pers.tile([B, 1], f32, tag="rs")
        nc.vector.reciprocal(out=rs[:], in_=sd[:])
        y = pers.tile([B, 2 * H], f32, tag="y")
        nc.vector.tensor_scalar(y[:], xc[:], rs[:], None, Alu.mult)
        if flags["ln_g"]:
            lng_sb = pers.tile([B, 2 * H], f32, tag="lng")
            nc.sync.dma_start(lng_sb[:], lng_d[:])
            nc.vector.tensor_tensor(out=y[:], in0=y[:], in1=lng_sb[:], op=Alu.mult)
        if flags["ln_b"]:
            lnb_sb = pers.tile([B, 2 * H], f32, tag="lnb")
            nc.sync.dma_start(lnb_sb[:], lnb_d[:])
            nc.vector.tensor_tensor(out=y[:], in0=y[:], in1=lnb_sb[:], op=Alu.add)
        h1_sb = pers.tile([B, 2 * H], f32, tag="h1")
        nc.scalar.activation(h1_sb[:], y[:], Act.Relu)

        # h1^T via PE transpose, then final matmul
        h1T = pers.tile([128, 2, B], f32, tag="h1T")
        pth = pp.tile([128, 2 * B], f32, tag="tp", bufs=2)
        for kc in range(2):
            nc.tensor.transpose(
                pth[:, kc * B:(kc + 1) * B],
                h1_sb[:, kc * 128:(kc + 1) * 128],
                ident[:B, :B],
            )
        nc.vector.tensor_copy(out=h1T[:].rearrange("p k b -> p (k b)"), in_=pth[:])

        po = pp.tile([B, H], f32, tag="gx")
        for kc in range(2):
            nc.tensor.matmul(
                po[:], lhsT=h1T[:, kc, :], rhs=w2_sb[:, kc, :],
                start=(kc == 0), stop=(kc == 1),
            )
        o_sb = pers.tile([B, H], f32, tag="osb")
        if flags["b2"]:
            b2_sb = pers.tile([B, H], f32, tag="b2")
            nc.sync.dma_start(b2_sb[:], b2_d[:])
            nc.vector.tensor_tensor(out=o_sb[:], in0=po[:], in1=b2_sb[:], op=Alu.add)
        else:
            nc.vector.tensor_copy(out=o_sb[:], in_=po[:])
        nc.sync.dma_start(out_d[:], o_sb[:])


def build_kernel(inputs):
    """Trace + compile; returns (nc, in_maps)."""
    from concourse import bacc, tile

    nodes = np.ascontiguousarray(
        np.asarray(inputs["nodes"], dtype=np.float32).reshape(N, 2 * H)
    )
    src = np.asarray(inputs["src"]).astype(np.int64)
    rel = np.asarray(inputs["rel"]).astype(np.int64)
    dst = np.asarray(inputs["dst"]).astype(np.int64)
    rel_W = np.ascontiguousarray(np.asarray(inputs["rel_W"], dtype=np.float32))
    rel_b = np.asarray(inputs["rel_b"], dtype=np.float32)
    gru_Wih = np.asarray(inputs["gru_Wih"], dtype=np.float32)
    gru_Whh = np.asarray(inputs["gru_Whh"], dtype=np.float32)
    gru_bih = np.asarray(inputs["gru_bih"], dtype=np.float32)
    gru_bhh = np.asarray(inputs["gru_bhh"], dtype=np.float32)
    mlp_W1 = np.ascontiguousarray(np.asarray(inputs["mlp_W1"], dtype=np.float32))
    mlp_b1 = np.asarray(inputs["mlp_b1"], dtype=np.float32)
    ln_g = np.asarray(inputs["ln_g"], dtype=np.float32)
    ln_b = np.asarray(inputs["ln_b"], dtype=np.float32)
    mlp_W2 = np.ascontiguousarray(np.asarray(inputs["mlp_W2"], dtype=np.float32))
    mlp_b2 = np.asarray(inputs["mlp_b2"], dtype=np.float32)
    tseq = np.ascontiguousarray(
        np.asarray(inputs["temporal_sequence"], dtype=np.float32).reshape(B * T, H)
    )

    # sigmoid-only GRU: tanh(x) = 2*sigmoid(2x) - 1, fold the 2x into n-gate
    # weights/biases (rows 2H:3H)
    wih2 = gru_Wih.copy()
    wih2[2 * H:] *= 2.0
    whh2 = gru_Whh.copy()
    whh2[2 * H:] *= 2.0
    bih2 = gru_bih.copy()
    bih2[2 * H:] *= 2.0
    bhh2 = gru_bhh.copy()
    bhh2[2 * H:] *= 2.0

    flags = {
        "rel_b": bool(np.any(rel_b != 0)),
        "bih": bool(np.any(bih2 != 0)),
        "bhh": bool(np.any(bhh2 != 0)),
        "ln_g": bool(np.any(ln_g != 1)),
        "ln_b": bool(np.any(ln_b != 0)),
        "b1": bool(np.any(mlp_b1 != 0)),
        "b2": bool(np.any(mlp_b2 != 0)),
    }

    nc = bacc.Bacc(
        "TRN2", target_bir_lowering=False, debug=False, num_devices=NCORES
    )
    with tile.TileContext(nc) as tc:
        _build(nc, tc, flags)
    nc.compile()

    in_maps = []
    for c in range(NCORES):
        lo, hi = c * EPC, (c + 1) * EPC
        s_pad = np.zeros(EPAD, np.int64)
        d_pad = np.zeros(EPAD, np.int64)
        r_pad = np.full(EPAD, R, np.int64)  # rel=R -> zero onehot row
        s_pad[:EPC] = src[lo:hi]
        d_pad[:EPC] = dst[lo:hi]
        r_pad[:EPC] = rel[lo:hi]
        m = {
            "nodes_flat": nodes,
            "sidx": _wrap_idx(s_pad),
            "didx": _wrap_idx(d_pad),
            "relv": np.ascontiguousarray(
                r_pad.reshape(TILES, 128).T.astype(np.float32)
            ),
            "relW": rel_W,
            "tseq": tseq,
            "wih": wih2,
            "whh": whh2,
            "w1": mlp_W1,
            "w2": mlp_W2,
        }
        if flags["rel_b"]:
            m["relb"] = rel_b
        if flags["bih"]:
            m["biht"] = np.ascontiguousarray(bih2.reshape(3, H).T)
        if flags["bhh"]:
            m["bhht"] = np.ascontiguousarray(bhh2.reshape(3, H).T)
        if flags["ln_g"]:
            m["lng"] = np.ascontiguousarray(np.broadcast_to(ln_g, (B, 2 * H)))
        if flags["ln_b"]:
            m["lnb"] = np.ascontiguousarray(np.broadcast_to(ln_b, (B, 2 * H)))
        if flags["b1"]:
            m["b1"] = np.ascontiguousarray(np.broadcast_to(mlp_b1, (B, 2 * H)))
        if flags["b2"]:
            m["b2"] = np.ascontiguousarray(np.broadcast_to(mlp_b2, (B, H)))
        in_maps.append(m)
    return nc, in_maps


def run(inputs, trace=False):
    from concourse import bass_utils

    nc, in_maps = build_kernel(inputs)
    res = bass_utils.run_bass_kernel_spmd(
        nc, in_maps, core_ids=list(range(NCORES)), trace=trace
    )
    return res


def kernel(**inputs):
    res = run(inputs, trace=False)
    return np.asarray(res.results[0]["out"], dtype=np.float32)
